# revision 5
# baseline (speedup 1.0000x reference)
"""Causal self-attention (GQA + RoPE) Trainium2 Bass kernel.

Sharding: 8 cores = batch(2) x kv-group(4). Each core computes its batch's
4 q-heads / 1 kv-head and a row-shard of the Wo projection; the 4 partial
outputs per batch are summed on host (all-reduce replacement).

Fused single-pass pipeline over 512-query rows: for each row n we
project+RoPE x block n, immediately run the causal attention row jq=n
(which only needs k/v blocks 0..4n+3, all available), normalize, and run
the Wo projection + y writeout for the row's 4 token blocks. This keeps
the PE dense (DVFS p-state ramps up), starts the softmax exps early, and
overlaps all DMA with compute.

q/k (post-RoPE), P (softmax probs), V, opk and Wo are bf16: same PE
cycles/row but no fp32r short-stream penalty, half the LDWEIGHTS cost and
SBUF traffic. S logits / rowsums / y accumulate in fp32.

Self-contained: hardcodes all shapes from the problem spec.
"""

import numpy as np

import concourse.bass as bass
import concourse.mybir as mybir
from concourse.tile import TileContext
from concourse.bass_utils import run_bass_kernel_spmd

F32 = mybir.dt.float32
F32R = mybir.dt.float32r
BF16 = mybir.dt.bfloat16

B, T, C = 2, 2048, 1024
H, HKV, D = 16, 4, 64
HALF = D // 2  # 32
GQ = H // HKV  # 4 q heads per group
FQ = GQ * D    # 256 q features per group
FPROJ = FQ + 2 * D  # 384: q(256) + k(64) + v(64)
NT = T // 512  # 4 row blocks of 512
KT = C // 128  # 8 contraction tiles
MT = FPROJ // 128  # 3 output row tiles (q01, q23, kv)
NEG = -1.0e9


def _split_excess_waits(nc, max_waits=1):
    """walrus here encodes at most one sync-wait per instruction; hoist the
    rest into standalone EventSemaphore instructions (raw-bass encoding)."""
    n = 0
    for fn in nc.m.functions:
        for bb in fn.blocks:
            new = []
            changed = False
            for inst in bb.instructions:
                si = inst.sync_info
                if si is not None and len(si.on_wait) > max_waits:
                    waits = list(si.on_wait)
                    for j, w in enumerate(waits[max_waits:]):
                        ev = mybir.InstEventSemaphore(
                            name=f"{inst.name}-ws{j}",
                            engine=inst.engine,
                            ins=[],
                            outs=[],
                            sync_info=mybir.SyncInfo(on_wait=[w], on_update=[]),
                        )
                        new.append(ev)
                        n += 1
                    inst.sync_info = mybir.SyncInfo(
                        on_wait=waits[:max_waits], on_update=list(si.on_update)
                    )
                    changed = True
                new.append(inst)
            if changed:
                bb.instructions = new
    return n


def _build():
    nc = bass.Bass()
    xt_d = nc.dram_tensor("xt", [C, T], F32, kind="ExternalInput")
    wproj_d = nc.dram_tensor("wproj", [C, FPROJ], F32, kind="ExternalInput")
    wo_d = nc.dram_tensor("wo", [FQ, C], BF16, kind="ExternalInput")
    atab_d = nc.dram_tensor("atab", [128, T], F32, kind="ExternalInput")
    btab_d = nc.dram_tensor("btab", [128, T], F32, kind="ExternalInput")
    pswap_d = nc.dram_tensor("pswap", [128, 128], F32, kind="ExternalInput")
    trib_d = nc.dram_tensor("trib", [128, 128], BF16, kind="ExternalInput")
    identb_d = nc.dram_tensor("identb", [128, 128], BF16, kind="ExternalInput")
    identr_d = nc.dram_tensor("identr", [64, 64], F32, kind="ExternalInput")
    ones65_d = nc.dram_tensor("ones65", [65, 64], F32, kind="ExternalInput")
    ones16_d = nc.dram_tensor("ones16", [128, 16], BF16, kind="ExternalInput")
    zeros_d = nc.dram_tensor("zeros64", [64, T], BF16, kind="ExternalInput")
    y_d = nc.dram_tensor("y", [T, C], F32, kind="ExternalOutput")

    xt_r = xt_d.rearrange("(ko p) t -> p ko t", p=128)
    wproj_r = wproj_d.rearrange("(ko p) f -> p ko f", p=128).bitcast(F32R)

    with TileContext(nc) as tc:
        from contextlib import ExitStack

        with ExitStack() as ctx:
            const = ctx.enter_context(tc.tile_pool(name="const", bufs=1))
            pers = ctx.enter_context(tc.tile_pool(name="pers", bufs=1))
            # --- constants ---
            wproj_sb = const.tile([128, KT, FPROJ], F32R)
            wo_sb = const.tile([128, 2, C], BF16)
            atab = const.tile([128, T], F32)
            btab = const.tile([128, T], F32)
            pswap = const.tile([128, 128], F32R)
            trib = const.tile([128, 128], BF16)
            identb = const.tile([128, 128], BF16)
            identr = const.tile([128, 64], F32R)
            ones65 = const.tile([65, 64], F32R)

            # --- persistent activations ---
            qr = [pers.tile([128, T], BF16, name=f"qr{i}") for i in range(2)]
            # k^T zero-padded to 128 contraction rows: kr0 = [k; 0] for even
            # heads, kr1 = [0; k] for odd heads -> S matmuls engage the full
            # PE array while the zero half kills the other head's q rows.
            kr0 = pers.tile([128, T], BF16)
            kr1 = pers.tile([128, T], BF16)
            vsb = pers.tile([128, T // 128, 65], BF16)  # v natural + ones col
            opk = pers.tile([128, 2, T], BF16)  # packed normalized O^T for Wo
            kvp = pers.tile([128, T], F32R)  # k^T rows 0:64, v^T rows 64:128

            xpool = ctx.enter_context(tc.tile_pool(name="xp", bufs=2))
            tmp = ctx.enter_context(tc.tile_pool(name="tmp", bufs=2))
            ppool = ctx.enter_context(tc.tile_pool(name="pp", bufs=4))
            o65pool = ctx.enter_context(tc.tile_pool(name="o65p", bufs=6))
            rcpool = ctx.enter_context(tc.tile_pool(name="rc", bufs=4))
            ypool = ctx.enter_context(tc.tile_pool(name="yp", bufs=2))
            # PSUM: pp(2) + s(2x2) + o(2) = 8 banks
            pp_ps = ctx.enter_context(
                tc.tile_pool(name="ppps", bufs=2, space="PSUM")
            )
            spool = ctx.enter_context(
                tc.tile_pool(name="sps", bufs=2, space="PSUM")
            )
            opool = ctx.enter_context(
                tc.tile_pool(name="ops", bufs=2, space="PSUM")
            )

            xrows = {}

            def emit_x_dma(n):
                xr = xpool.tile([128, KT, 512], F32R, tag="x", name=f"x{n}")
                xrows[n] = xr
                if n == 0:
                    return  # row 0 loads per-k, interleaved with wproj
                for half in range(2):
                    ks = slice(4 * half, 4 * half + 4)
                    nc.sync.dma_start(
                        xr[:, ks], xt_r[:, ks, bass.ts(n, 512)].bitcast(F32R)
                    )

            def emit_preamble_dma():
                emit_x_dma(0)
                xr = xrows[0]
                for k in range(KT):
                    nc.sync.dma_start(wproj_sb[:, k], wproj_r[:, k])
                    nc.sync.dma_start(
                        xr[:, k], xt_r[:, k, bass.ts(0, 512)].bitcast(F32R)
                    )
                nc.sync.dma_start(atab[:], atab_d[:])
                nc.sync.dma_start(btab[:], btab_d[:])
                nc.sync.dma_start(pswap[:], pswap_d[:].bitcast(F32R))
                nc.sync.dma_start(identr[64:128, :], identr_d[:].bitcast(F32R))
                nc.sync.dma_start(kr0[64:128, :], zeros_d[:])
                nc.sync.dma_start(kr1[0:64, :], zeros_d[:])
                nc.sync.dma_start(vsb[:, :, 64], ones16_d[:])
                nc.sync.dma_start(trib[:], trib_d[:])
                nc.sync.dma_start(identb[:], identb_d[:])
                nc.sync.dma_start(ones65[:], ones65_d[:].bitcast(F32R))
                nc.sync.dma_start(
                    wo_sb[:], wo_d.rearrange("(ko p) c -> p ko c", p=128)
                )

            def emit_proj(n):
                """projections + RoPE for token block n -> qr/kr/vsb cols."""
                xr = xrows[n]
                for m in range(MT):
                    ps = pp_ps.tile([128, 512], F32, tag="pp", name=f"ps{m}")
                    for k in range(KT):
                        nc.tensor.matmul(
                            ps[:],
                            wproj_sb[:, k, bass.ts(m, 128)],
                            xr[:, k],
                            start=(k == 0),
                            stop=(k == KT - 1),
                        )
                    rows = 128 if m < 2 else 64
                    if m == 2:
                        plain = kvp[:, bass.ts(n, 512)]
                    else:
                        qt_t = tmp.tile([128, 512], F32R, tag="qt", name="qt")
                        plain = qt_t[:]
                    nc.vector.tensor_copy(plain, ps[:])
                    qsw = pp_ps.tile([128, 512], F32, tag="pp", name=f"qsw{m}")
                    nc.tensor.matmul(
                        qsw[0:rows],
                        pswap[0:rows, 0:rows],
                        plain[0:rows],
                        start=True,
                        stop=True,
                    )
                    t1 = tmp.tile([128, 512], F32, tag="t1")
                    nc.vector.tensor_tensor(
                        t1[0:rows],
                        plain[0:rows].bitcast(F32),
                        atab[0:rows, bass.ts(n, 512)],
                        mybir.AluOpType.mult,
                    )
                    t2 = tmp.tile([128, 512], F32, tag="t2")
                    nc.vector.tensor_tensor(
                        t2[0:rows],
                        qsw[0:rows],
                        btab[0:rows, bass.ts(n, 512)],
                        mybir.AluOpType.mult,
                    )
                    dest = qr[m] if m < 2 else kr0
                    nc.gpsimd.tensor_tensor(
                        dest[0:rows, bass.ts(n, 512)],
                        t1[0:rows],
                        t2[0:rows],
                        mybir.AluOpType.add,
                    )
                    if m == 2:
                        # duplicate k^T into kr1 rows 64:128
                        nc.vector.tensor_copy(
                            kr1[64:128, bass.ts(n, 512)],
                            kr0[0:64, bass.ts(n, 512)],
                        )
                        # v^T -> v natural (PE transpose per 128-token block)
                        for tt in range(4 * n, 4 * n + 4):
                            vt_ps = pp_ps.tile(
                                [128, 64], F32, tag="pp", name="vt"
                            )
                            nc.tensor.transpose(
                                vt_ps[:],
                                kvp[64:128, bass.ts(tt, 128)].bitcast(F32),
                                identr[64:128, :].bitcast(F32),
                            )
                            nc.vector.tensor_copy(vsb[:, tt, 0:64], vt_ps[:])

            def emit_attn_row(n):
                """causal attention for query block n (all 4 heads)."""
                jq = n
                nkb = 4 * (jq + 1)
                o65s = {}
                for hp in range(2):
                    qtile = qr[hp]
                    heads = (2 * hp, 2 * hp + 1)
                    o_ps = {
                        h: opool.tile([65, 512], F32, tag="o", name=f"o{h}")
                        for h in heads
                    }
                    pend = None
                    for ksb in range(nkb // 2):
                        regions = []
                        for jk in range(2):
                            kb = 2 * ksb + jk
                            j = kb - 4 * jq
                            col0 = max(j, 0) * 128
                            regions.append((jk, col0, kb))
                        s_ps = {}
                        p_sb = {}
                        for h in heads:
                            s_ps[h] = spool.tile(
                                [128, 1024], F32, tag="s", name=f"s{h}"
                            )
                            p_sb[h] = ppool.tile(
                                [128, 1024], BF16, tag="p", name=f"pb{h}"
                            )
                        for jk, col0, kb in regions:
                            for h in heads:
                                krt = kr0 if h % 2 == 0 else kr1
                                nc.tensor.matmul(
                                    s_ps[h][:, jk * 512 + col0 : jk * 512 + 512],
                                    krt[:, bass.ts(kb, 128)],
                                    qtile[:, jq * 512 + col0 : jq * 512 + 512],
                                    start=True,
                                    stop=(kb < 4 * jq),
                                )
                        for h in heads:
                            for jk, col0, kb in regions:
                                if kb - 4 * jq >= 0:
                                    nc.tensor.matmul(
                                        s_ps[h][
                                            :,
                                            jk * 512 + col0 : jk * 512 + col0 + 128,
                                        ],
                                        identb[:],
                                        trib[:],
                                        start=False,
                                        stop=True,
                                    )
                        for h in heads:
                            if regions[0][1] == 0 and regions[1][1] == 0:
                                nc.scalar.activation(
                                    p_sb[h][:],
                                    s_ps[h][:],
                                    mybir.ActivationFunctionType.Exp,
                                    scale=0.125,
                                )
                            else:
                                for jk, col0, kb in regions:
                                    nc.scalar.activation(
                                        p_sb[h][:, jk * 512 + col0 : jk * 512 + 512],
                                        s_ps[h][:, jk * 512 + col0 : jk * 512 + 512],
                                        mybir.ActivationFunctionType.Exp,
                                        scale=0.125,
                                    )
                        # O matmuls run one K-step behind S so the PE queue
                        # never head-of-line blocks on the scalar exps.
                        if pend is not None:
                            for h in heads:
                                for jk, col0, kb in pend[0]:
                                    nc.tensor.matmul(
                                        o_ps[h][:, col0:512],
                                        vsb[:, kb, :],
                                        pend[1][h][
                                            :, jk * 512 + col0 : jk * 512 + 512
                                        ],
                                        start=(kb == 0),
                                        stop=(kb == nkb - 1),
                                    )
                        pend = (regions, p_sb)
                    for h in heads:
                        for jk, col0, kb in pend[0]:
                            nc.tensor.matmul(
                                o_ps[h][:, col0:512],
                                vsb[:, kb, :],
                                pend[1][h][:, jk * 512 + col0 : jk * 512 + 512],
                                start=(kb == 0),
                                stop=(kb == nkb - 1),
                            )
                    for h in heads:
                        o65t = o65pool.tile(
                            [65, 512], F32R, tag="o65", name=f"o65_{h}_{jq}"
                        )
                        nc.vector.tensor_copy(o65t[:], o_ps[h][:])
                        o65s[h] = o65t
                # normalization: 1/rowsum via DVE recip, broadcast via PE,
                # apply via DVE -> packed bf16 O^T in opk
                for h in range(4):
                    o65t = o65s[h]
                    rcf = rcpool.tile([1, 512], F32, tag="rcf", name=f"rcf{h}")
                    nc.vector.reciprocal(rcf[0:1, :], o65t[64:65, :].bitcast(F32))
                    rc = rcpool.tile([1, 512], F32R, tag="rc", name=f"rc{h}")
                    nc.vector.tensor_copy(rc[0:1, :], rcf[0:1, :])
                    bc_ps = pp_ps.tile([64, 512], F32, tag="pp", name=f"bc{h}")
                    nc.tensor.matmul(
                        bc_ps[:],
                        ones65[0:1, :],
                        rc[0:1, :],
                        start=True,
                        stop=True,
                    )
                    nc.vector.tensor_tensor(
                        opk[(h % 2) * 64 : (h % 2) * 64 + 64, h // 2, bass.ts(jq, 512)],
                        o65t[0:64, :].bitcast(F32),
                        bc_ps[:],
                        mybir.AluOpType.mult,
                    )

            def emit_wo(n):
                """Wo projection + writeout for the row's 4 token blocks."""
                for t in range(4 * n, 4 * n + 4):
                    y_sb = ypool.tile([128, C], F32, tag="y", name="ysb")
                    for nn in range(2):
                        wps = pp_ps.tile([128, 512], F32, tag="pp", name="wps")
                        for k in range(2):
                            nc.tensor.matmul(
                                wps[:],
                                opk[:, k, bass.ts(t, 128)],
                                wo_sb[:, k, bass.ts(nn, 512)],
                                start=(k == 0),
                                stop=(k == 1),
                            )
                        if (t + nn) % 2 == 0:
                            nc.vector.tensor_copy(
                                y_sb[:, bass.ts(nn, 512)], wps[:]
                            )
                        else:
                            nc.scalar.copy(y_sb[:, bass.ts(nn, 512)], wps[:])
                    nc.sync.dma_start(y_d[bass.ts(t, 128), :], y_sb[:])

            emit_preamble_dma()
            for n in range(NT):
                emit_proj(n)
                if n + 1 < NT:
                    emit_x_dma(n + 1)
                emit_attn_row(n)
                emit_wo(n)

    _split_excess_waits(nc)
    return nc


_NC_CACHE = None


def _get_nc():
    global _NC_CACHE
    if _NC_CACHE is None:
        _NC_CACHE = _build()
    return _NC_CACHE


def _host_prep(x, cos, sin, Wq, Wk, Wv, Wo):
    import ml_dtypes

    cos2 = np.asarray(cos, np.float32).reshape(T, HALF)  # [T, 32]
    sin2 = np.asarray(sin, np.float32).reshape(T, HALF)
    atab = np.tile(cos2.T, (4, 1))  # [128, T]
    btab = np.tile(np.vstack([sin2.T, -sin2.T]), (2, 1))  # [128, T]
    idx = np.arange(128)
    pswap = np.zeros((128, 128), np.float32)
    pswap[idx ^ 32, idx] = 1.0
    k_i = np.arange(128)[:, None]
    q_i = np.arange(128)[None, :]
    trib = np.where(k_i > q_i, np.float32(NEG), np.float32(0.0)).astype(
        ml_dtypes.bfloat16
    )
    identb = np.eye(128, dtype=ml_dtypes.bfloat16)
    identr = np.eye(64, dtype=np.float32)
    ones65 = np.ones((65, 64), np.float32)
    ones16 = np.ones((128, 16), ml_dtypes.bfloat16)
    zeros64 = np.zeros((64, T), ml_dtypes.bfloat16)

    in_maps = []
    for core in range(8):
        b, g = core // 4, core % 4
        xt = np.ascontiguousarray(np.asarray(x[b], np.float32).T)  # [C, T]
        wproj = np.ascontiguousarray(
            np.concatenate(
                [
                    Wq[:, g * FQ : (g + 1) * FQ],
                    Wk[:, g * D : (g + 1) * D],
                    Wv[:, g * D : (g + 1) * D],
                ],
                axis=1,
            ).astype(np.float32)
        )
        wo = np.ascontiguousarray(
            Wo[g * FQ : (g + 1) * FQ, :].astype(ml_dtypes.bfloat16)
        )
        in_maps.append(
            {
                "xt": xt,
                "wproj": wproj,
                "wo": wo,
                "atab": atab,
                "btab": btab,
                "pswap": pswap,
                "trib": trib,
                "identb": identb,
                "identr": identr,
                "ones65": ones65,
                "ones16": ones16,
                "zeros64": zeros64,
            }
        )
    return in_maps


def kernel(x, cos, sin, Wq, Wk, Wv, Wo, _want_trace=False, _trace_kwargs=None):
    nc = _get_nc()
    in_maps = _host_prep(x, cos, sin, Wq, Wk, Wv, Wo)
    kw = {}
    if _want_trace:
        kw = dict(trace=True, **(_trace_kwargs or {}))
    res = run_bass_kernel_spmd(nc, in_maps, list(range(8)), **kw)
    y = np.zeros((B, T, C), np.float32)
    for core in range(8):
        b = core // 4
        y[b] += res.results[core]["y"]
    if _want_trace:
        kernel.last_result = res
    return y


# revision 9
# speedup vs baseline: 1.3326x; 1.3326x over previous
"""Causal self-attention (GQA + RoPE) Trainium2 Bass kernel.

Sharding: 8 cores = batch(2) x kv-group(4). Each core computes its batch's
4 q-heads / 1 kv-head and a row-shard of the Wo projection; the 4 partial
outputs per batch are summed on host (all-reduce replacement).

Fused single-pass pipeline over 512-query rows: for each row n we
project+RoPE x block n, immediately run the causal attention row jq=n
(which only needs k/v blocks 0..4n+3, all available), normalize, and run
the Wo projection + y writeout for the row's 4 token blocks. This keeps
the PE dense (DVFS p-state ramps up), starts the softmax exps early, and
overlaps all DMA with compute.

q/k (post-RoPE), P (softmax probs), V, opk and Wo are bf16: same PE
cycles/row but no fp32r short-stream penalty, half the LDWEIGHTS cost and
SBUF traffic. S logits / rowsums / y accumulate in fp32.

Self-contained: hardcodes all shapes from the problem spec.
"""

import numpy as np

import concourse.bass as bass
import concourse.mybir as mybir
from concourse.tile import TileContext
from concourse.bass_utils import run_bass_kernel_spmd

F32 = mybir.dt.float32
F32R = mybir.dt.float32r
BF16 = mybir.dt.bfloat16

B, T, C = 2, 2048, 1024
H, HKV, D = 16, 4, 64
HALF = D // 2  # 32
GQ = H // HKV  # 4 q heads per group
FQ = GQ * D    # 256 q features per group
FPROJ = FQ + 2 * D  # 384: q(256) + k(64) + v(64)
NT = T // 512  # 4 row blocks of 512
KT = C // 128  # 8 contraction tiles
MT = FPROJ // 128  # 3 output row tiles (q01, q23, kv)
NEG = -1.0e9


def _split_excess_waits(nc, max_waits=1):
    """walrus here encodes at most one sync-wait per instruction; hoist the
    rest into standalone EventSemaphore instructions (raw-bass encoding)."""
    n = 0
    for fn in nc.m.functions:
        for bb in fn.blocks:
            new = []
            changed = False
            for inst in bb.instructions:
                si = inst.sync_info
                if si is not None and len(si.on_wait) > max_waits:
                    waits = list(si.on_wait)
                    for j, w in enumerate(waits[max_waits:]):
                        ev = mybir.InstEventSemaphore(
                            name=f"{inst.name}-ws{j}",
                            engine=inst.engine,
                            ins=[],
                            outs=[],
                            sync_info=mybir.SyncInfo(on_wait=[w], on_update=[]),
                        )
                        new.append(ev)
                        n += 1
                    inst.sync_info = mybir.SyncInfo(
                        on_wait=waits[:max_waits], on_update=list(si.on_update)
                    )
                    changed = True
                new.append(inst)
            if changed:
                bb.instructions = new
    return n


def _build():
    nc = bass.Bass()
    xt_d = nc.dram_tensor("xt", [C, T], F32, kind="ExternalInput")
    wproj_d = nc.dram_tensor("wproj", [C, FPROJ], F32, kind="ExternalInput")
    wo_d = nc.dram_tensor("wo", [FQ, C], BF16, kind="ExternalInput")
    atab_d = nc.dram_tensor("atab", [128, T], F32, kind="ExternalInput")
    btab_d = nc.dram_tensor("btab", [128, T], F32, kind="ExternalInput")
    pswap_d = nc.dram_tensor("pswap", [128, 128], F32, kind="ExternalInput")
    trib_d = nc.dram_tensor("trib", [128, 128], BF16, kind="ExternalInput")
    identb_d = nc.dram_tensor("identb", [128, 128], BF16, kind="ExternalInput")
    identr_d = nc.dram_tensor("identr", [64, 64], F32, kind="ExternalInput")
    ones65_d = nc.dram_tensor("ones65", [65, 64], F32, kind="ExternalInput")
    ones16_d = nc.dram_tensor("ones16", [128, 16], BF16, kind="ExternalInput")
    zeros_d = nc.dram_tensor("zeros64", [64, T], BF16, kind="ExternalInput")
    y_d = nc.dram_tensor("y", [T, C], F32, kind="ExternalOutput")

    xt_r = xt_d.rearrange("(ko p) t -> p ko t", p=128)
    wproj_r = wproj_d.rearrange("(ko p) f -> p ko f", p=128).bitcast(F32R)

    with TileContext(nc) as tc:
        from contextlib import ExitStack

        with ExitStack() as ctx:
            const = ctx.enter_context(tc.tile_pool(name="const", bufs=1))
            pers = ctx.enter_context(tc.tile_pool(name="pers", bufs=1))
            # --- constants ---
            wproj_sb = const.tile([128, KT, FPROJ], F32R)
            wo_sb = const.tile([128, 2, C], BF16)
            atab = const.tile([128, T], F32)
            btab = const.tile([128, T], F32)
            pswap = const.tile([128, 128], F32R)
            trib = const.tile([128, 128], BF16)
            identb = const.tile([128, 128], BF16)
            identr = const.tile([128, 64], F32R)
            ones65 = const.tile([65, 64], F32R)

            # --- persistent activations ---
            qr = [pers.tile([128, T], BF16, name=f"qr{i}") for i in range(2)]
            # k^T zero-padded to 128 contraction rows: kr0 = [k; 0] for even
            # heads, kr1 = [0; k] for odd heads -> S matmuls engage the full
            # PE array while the zero half kills the other head's q rows.
            kr0 = pers.tile([128, T], BF16)
            kr1 = pers.tile([128, T], BF16)
            vsb = pers.tile([128, T // 128, 65], BF16)  # v natural + ones col
            opk = pers.tile([128, 2, T], BF16)  # packed normalized O^T for Wo
            kvp = pers.tile([128, T], F32R)  # k^T rows 0:64, v^T rows 64:128

            xpool = ctx.enter_context(tc.tile_pool(name="xp", bufs=2))
            tmp = ctx.enter_context(tc.tile_pool(name="tmp", bufs=2))
            ppool = ctx.enter_context(tc.tile_pool(name="pp", bufs=4))
            o65pool = ctx.enter_context(tc.tile_pool(name="o65p", bufs=2))
            rcpool = ctx.enter_context(tc.tile_pool(name="rc", bufs=2))
            ypool = ctx.enter_context(tc.tile_pool(name="yp", bufs=2))
            # PSUM: pp(2) + s(2x2) + o(2) = 8 banks
            pp_ps = ctx.enter_context(
                tc.tile_pool(name="ppps", bufs=2, space="PSUM")
            )
            spool = ctx.enter_context(
                tc.tile_pool(name="sps", bufs=2, space="PSUM")
            )
            opool = ctx.enter_context(
                tc.tile_pool(name="ops", bufs=2, space="PSUM")
            )

            xrows = {}

            def emit_x_dma(n):
                xr = xpool.tile([128, KT, 512], F32R, tag="x", name=f"x{n}")
                xrows[n] = xr
                if n == 0:
                    return  # row 0 loads per-k, interleaved with wproj
                for half in range(2):
                    ks = slice(4 * half, 4 * half + 4)
                    nc.sync.dma_start(
                        xr[:, ks], xt_r[:, ks, bass.ts(n, 512)].bitcast(F32R)
                    )

            def emit_preamble_dma():
                emit_x_dma(0)
                xr = xrows[0]
                for k in range(KT):
                    nc.sync.dma_start(wproj_sb[:, k], wproj_r[:, k])
                    nc.sync.dma_start(
                        xr[:, k], xt_r[:, k, bass.ts(0, 512)].bitcast(F32R)
                    )
                nc.sync.dma_start(atab[:], atab_d[:])
                nc.sync.dma_start(btab[:], btab_d[:])
                nc.sync.dma_start(pswap[:], pswap_d[:].bitcast(F32R))
                nc.sync.dma_start(identr[64:128, :], identr_d[:].bitcast(F32R))
                nc.sync.dma_start(kr0[64:128, :], zeros_d[:])
                nc.sync.dma_start(kr1[0:64, :], zeros_d[:])
                nc.sync.dma_start(vsb[:, :, 64], ones16_d[:])
                nc.sync.dma_start(trib[:], trib_d[:])
                nc.sync.dma_start(identb[:], identb_d[:])
                nc.sync.dma_start(ones65[:], ones65_d[:].bitcast(F32R))
                nc.sync.dma_start(
                    wo_sb[:], wo_d.rearrange("(ko p) c -> p ko c", p=128)
                )

            def emit_proj(n):
                """projections + RoPE for token block n -> qr/kr/vsb cols."""
                xr = xrows[n]
                for m in range(MT):
                    ps = pp_ps.tile([128, 512], F32, tag="pp", name=f"ps{m}")
                    for k in range(KT):
                        nc.tensor.matmul(
                            ps[:],
                            wproj_sb[:, k, bass.ts(m, 128)],
                            xr[:, k],
                            start=(k == 0),
                            stop=(k == KT - 1),
                        )
                    rows = 128 if m < 2 else 64
                    if m == 2:
                        plain = kvp[:, bass.ts(n, 512)]
                    else:
                        qt_t = tmp.tile([128, 512], F32R, tag="qt", name="qt")
                        plain = qt_t[:]
                    nc.vector.tensor_copy(plain, ps[:])
                    qsw = pp_ps.tile([128, 512], F32, tag="pp", name=f"qsw{m}")
                    nc.tensor.matmul(
                        qsw[0:rows],
                        pswap[0:rows, 0:rows],
                        plain[0:rows],
                        start=True,
                        stop=True,
                    )
                    t1 = tmp.tile([128, 512], F32, tag="t1")
                    nc.vector.tensor_tensor(
                        t1[0:rows],
                        plain[0:rows].bitcast(F32),
                        atab[0:rows, bass.ts(n, 512)],
                        mybir.AluOpType.mult,
                    )
                    t2 = tmp.tile([128, 512], F32, tag="t2")
                    nc.vector.tensor_tensor(
                        t2[0:rows],
                        qsw[0:rows],
                        btab[0:rows, bass.ts(n, 512)],
                        mybir.AluOpType.mult,
                    )
                    dest = qr[m] if m < 2 else kr0
                    nc.gpsimd.tensor_tensor(
                        dest[0:rows, bass.ts(n, 512)],
                        t1[0:rows],
                        t2[0:rows],
                        mybir.AluOpType.add,
                    )
                    if m == 2:
                        # duplicate k^T into kr1 rows 64:128
                        nc.vector.tensor_copy(
                            kr1[64:128, bass.ts(n, 512)],
                            kr0[0:64, bass.ts(n, 512)],
                        )
                        # v^T -> v natural (PE transpose per 128-token block)
                        for tt in range(4 * n, 4 * n + 4):
                            vt_ps = pp_ps.tile(
                                [128, 64], F32, tag="pp", name="vt"
                            )
                            nc.tensor.transpose(
                                vt_ps[:],
                                kvp[64:128, bass.ts(tt, 128)].bitcast(F32),
                                identr[64:128, :].bitcast(F32),
                            )
                            nc.vector.tensor_copy(vsb[:, tt, 0:64], vt_ps[:])

            def emit_attn_row(n):
                """causal attention for query block n (all 4 heads)."""
                jq = n
                nkb = 4 * (jq + 1)
                o65b = o65pool.tile(
                    [65, 4, 512], F32R, tag="o65", name=f"o65_{jq}"
                )
                for hp in range(2):
                    qtile = qr[hp]
                    heads = (2 * hp, 2 * hp + 1)
                    o_ps = {
                        h: opool.tile([65, 512], F32, tag="o", name=f"o{h}")
                        for h in heads
                    }
                    pend = None
                    for ksb in range(nkb // 2):
                        regions = []
                        for jk in range(2):
                            kb = 2 * ksb + jk
                            j = kb - 4 * jq
                            col0 = max(j, 0) * 128
                            regions.append((jk, col0, kb))
                        s_ps = {}
                        p_sb = {}
                        for h in heads:
                            s_ps[h] = spool.tile(
                                [128, 1024], F32, tag="s", name=f"s{h}"
                            )
                            p_sb[h] = ppool.tile(
                                [128, 1024], BF16, tag="p", name=f"pb{h}"
                            )
                        for jk, col0, kb in regions:
                            for h in heads:
                                krt = kr0 if h % 2 == 0 else kr1
                                nc.tensor.matmul(
                                    s_ps[h][:, jk * 512 + col0 : jk * 512 + 512],
                                    krt[:, bass.ts(kb, 128)],
                                    qtile[:, jq * 512 + col0 : jq * 512 + 512],
                                    start=True,
                                    stop=(kb < 4 * jq),
                                )
                        for h in heads:
                            for jk, col0, kb in regions:
                                if kb - 4 * jq >= 0:
                                    nc.tensor.matmul(
                                        s_ps[h][
                                            :,
                                            jk * 512 + col0 : jk * 512 + col0 + 128,
                                        ],
                                        identb[:],
                                        trib[:],
                                        start=False,
                                        stop=True,
                                    )
                        for h in heads:
                            if regions[0][1] == 0 and regions[1][1] == 0:
                                nc.scalar.activation(
                                    p_sb[h][:],
                                    s_ps[h][:],
                                    mybir.ActivationFunctionType.Exp,
                                    scale=0.125,
                                )
                            else:
                                for jk, col0, kb in regions:
                                    nc.scalar.activation(
                                        p_sb[h][:, jk * 512 + col0 : jk * 512 + 512],
                                        s_ps[h][:, jk * 512 + col0 : jk * 512 + 512],
                                        mybir.ActivationFunctionType.Exp,
                                        scale=0.125,
                                    )
                        # O matmuls run one K-step behind S so the PE queue
                        # never head-of-line blocks on the scalar exps.
                        if pend is not None:
                            for h in heads:
                                for jk, col0, kb in pend[0]:
                                    nc.tensor.matmul(
                                        o_ps[h][:, col0:512],
                                        vsb[:, kb, :],
                                        pend[1][h][
                                            :, jk * 512 + col0 : jk * 512 + 512
                                        ],
                                        start=(kb == 0),
                                        stop=(kb == nkb - 1),
                                    )
                        pend = (regions, p_sb)
                    for h in heads:
                        for jk, col0, kb in pend[0]:
                            nc.tensor.matmul(
                                o_ps[h][:, col0:512],
                                vsb[:, kb, :],
                                pend[1][h][:, jk * 512 + col0 : jk * 512 + 512],
                                start=(kb == 0),
                                stop=(kb == nkb - 1),
                            )
                    for h in heads:
                        nc.vector.tensor_copy(o65b[:, h, :], o_ps[h][:])
                # normalization: 1/rowsum via batched Ln + Exp(-x) on the
                # scalar engine (both live in the natural_log_exp table, so
                # no act-table reloads), broadcast via PE, apply via DVE.
                lnd = rcpool.tile(
                    [1, 4, 512], F32, tag="lnd", name=f"ln{jq}", bufs=1
                )
                nc.scalar.activation(
                    lnd[0:1, :, :],
                    o65b[64:65, :, :].bitcast(F32),
                    mybir.ActivationFunctionType.Ln,
                )
                rc = rcpool.tile([1, 4, 512], F32R, tag="rc", name=f"rc{jq}")
                nc.scalar.activation(
                    rc[0:1, :, :],
                    lnd[0:1, :, :],
                    mybir.ActivationFunctionType.Exp,
                    scale=-1.0,
                )
                for h in range(4):
                    bc_ps = pp_ps.tile([64, 512], F32, tag="pp", name=f"bc{h}")
                    nc.tensor.matmul(
                        bc_ps[:],
                        ones65[0:1, :],
                        rc[0:1, h, :],
                        start=True,
                        stop=True,
                    )
                    nc.vector.tensor_tensor(
                        opk[(h % 2) * 64 : (h % 2) * 64 + 64, h // 2, bass.ts(jq, 512)],
                        o65b[0:64, h, :].bitcast(F32),
                        bc_ps[:],
                        mybir.AluOpType.mult,
                    )

            def emit_wo(n):
                """Wo projection + writeout for the row's 4 token blocks."""
                for t in range(4 * n, 4 * n + 4):
                    y_sb = ypool.tile([128, C], F32, tag="y", name="ysb")
                    for nn in range(2):
                        wps = pp_ps.tile([128, 512], F32, tag="pp", name="wps")
                        for k in range(2):
                            nc.tensor.matmul(
                                wps[:],
                                opk[:, k, bass.ts(t, 128)],
                                wo_sb[:, k, bass.ts(nn, 512)],
                                start=(k == 0),
                                stop=(k == 1),
                            )
                        if (t + nn) % 2 == 0:
                            nc.vector.tensor_copy(
                                y_sb[:, bass.ts(nn, 512)], wps[:]
                            )
                        else:
                            nc.scalar.copy(y_sb[:, bass.ts(nn, 512)], wps[:])
                    nc.sync.dma_start(y_d[bass.ts(t, 128), :], y_sb[:])

            emit_preamble_dma()
            for n in range(NT):
                emit_proj(n)
                if n + 1 < NT:
                    emit_x_dma(n + 1)
                emit_attn_row(n)
                emit_wo(n)

    _split_excess_waits(nc)
    return nc


_NC_CACHE = None


def _get_nc():
    global _NC_CACHE
    if _NC_CACHE is None:
        _NC_CACHE = _build()
    return _NC_CACHE


def _host_prep(x, cos, sin, Wq, Wk, Wv, Wo):
    import ml_dtypes

    cos2 = np.asarray(cos, np.float32).reshape(T, HALF)  # [T, 32]
    sin2 = np.asarray(sin, np.float32).reshape(T, HALF)
    atab = np.tile(cos2.T, (4, 1))  # [128, T]
    btab = np.tile(np.vstack([sin2.T, -sin2.T]), (2, 1))  # [128, T]
    idx = np.arange(128)
    pswap = np.zeros((128, 128), np.float32)
    pswap[idx ^ 32, idx] = 1.0
    k_i = np.arange(128)[:, None]
    q_i = np.arange(128)[None, :]
    trib = np.where(k_i > q_i, np.float32(NEG), np.float32(0.0)).astype(
        ml_dtypes.bfloat16
    )
    identb = np.eye(128, dtype=ml_dtypes.bfloat16)
    identr = np.eye(64, dtype=np.float32)
    ones65 = np.ones((65, 64), np.float32)
    ones16 = np.ones((128, 16), ml_dtypes.bfloat16)
    zeros64 = np.zeros((64, T), ml_dtypes.bfloat16)

    in_maps = []
    for core in range(8):
        b, g = core // 4, core % 4
        xt = np.ascontiguousarray(np.asarray(x[b], np.float32).T)  # [C, T]
        wproj = np.ascontiguousarray(
            np.concatenate(
                [
                    Wq[:, g * FQ : (g + 1) * FQ],
                    Wk[:, g * D : (g + 1) * D],
                    Wv[:, g * D : (g + 1) * D],
                ],
                axis=1,
            ).astype(np.float32)
        )
        wo = np.ascontiguousarray(
            Wo[g * FQ : (g + 1) * FQ, :].astype(ml_dtypes.bfloat16)
        )
        in_maps.append(
            {
                "xt": xt,
                "wproj": wproj,
                "wo": wo,
                "atab": atab,
                "btab": btab,
                "pswap": pswap,
                "trib": trib,
                "identb": identb,
                "identr": identr,
                "ones65": ones65,
                "ones16": ones16,
                "zeros64": zeros64,
            }
        )
    return in_maps


def kernel(x, cos, sin, Wq, Wk, Wv, Wo, _want_trace=False, _trace_kwargs=None):
    nc = _get_nc()
    in_maps = _host_prep(x, cos, sin, Wq, Wk, Wv, Wo)
    kw = {}
    if _want_trace:
        kw = dict(trace=True, **(_trace_kwargs or {}))
    res = run_bass_kernel_spmd(nc, in_maps, list(range(8)), **kw)
    y = np.zeros((B, T, C), np.float32)
    for core in range(8):
        b = core // 4
        y[b] += res.results[core]["y"]
    if _want_trace:
        kernel.last_result = res
    return y


# revision 10
# speedup vs baseline: 1.5538x; 1.1660x over previous
"""Causal self-attention (GQA + RoPE) Trainium2 Bass kernel.

Sharding: 8 cores = batch(2) x kv-group(4). Each core computes its batch's
4 q-heads / 1 kv-head and a row-shard of the Wo projection; the 4 partial
outputs per batch are summed on host (all-reduce replacement).

Fused single-pass pipeline over 512-query rows: for each row n we
project+RoPE x block n, immediately run the causal attention row jq=n
(which only needs k/v blocks 0..4n+3, all available), normalize, and run
the Wo projection + y writeout for the row's 4 token blocks. This keeps
the PE dense (DVFS p-state ramps up), starts the softmax exps early, and
overlaps all DMA with compute.

q/k (post-RoPE), P (softmax probs), V, opk and Wo are bf16: same PE
cycles/row but no fp32r short-stream penalty, half the LDWEIGHTS cost and
SBUF traffic. S logits / rowsums / y accumulate in fp32.

Self-contained: hardcodes all shapes from the problem spec.
"""

import numpy as np

import concourse.bass as bass
import concourse.mybir as mybir
from concourse.tile import TileContext
from concourse.bass_utils import run_bass_kernel_spmd

F32 = mybir.dt.float32
F32R = mybir.dt.float32r
BF16 = mybir.dt.bfloat16

B, T, C = 2, 2048, 1024
H, HKV, D = 16, 4, 64
HALF = D // 2  # 32
GQ = H // HKV  # 4 q heads per group
FQ = GQ * D    # 256 q features per group
FPROJ = FQ + 2 * D  # 384: q(256) + k(64) + v(64)
NT = T // 512  # 4 row blocks of 512
KT = C // 128  # 8 contraction tiles
MT = FPROJ // 128  # 3 output row tiles (q01, q23, kv)
NEG = -1.0e9


def _split_excess_waits(nc, max_waits=1):
    """walrus here encodes at most one sync-wait per instruction; hoist the
    rest into standalone EventSemaphore instructions (raw-bass encoding)."""
    n = 0
    for fn in nc.m.functions:
        for bb in fn.blocks:
            new = []
            changed = False
            for inst in bb.instructions:
                si = inst.sync_info
                if si is not None and len(si.on_wait) > max_waits:
                    waits = list(si.on_wait)
                    for j, w in enumerate(waits[max_waits:]):
                        ev = mybir.InstEventSemaphore(
                            name=f"{inst.name}-ws{j}",
                            engine=inst.engine,
                            ins=[],
                            outs=[],
                            sync_info=mybir.SyncInfo(on_wait=[w], on_update=[]),
                        )
                        new.append(ev)
                        n += 1
                    inst.sync_info = mybir.SyncInfo(
                        on_wait=waits[:max_waits], on_update=list(si.on_update)
                    )
                    changed = True
                new.append(inst)
            if changed:
                bb.instructions = new
    return n


def _build():
    nc = bass.Bass()
    xt_d = nc.dram_tensor("xt", [C, T], F32, kind="ExternalInput")
    wproj_d = nc.dram_tensor("wproj", [C, FPROJ], F32, kind="ExternalInput")
    wo_d = nc.dram_tensor("wo", [FQ, C], BF16, kind="ExternalInput")
    atab_d = nc.dram_tensor("atab", [128, T], F32, kind="ExternalInput")
    btab_d = nc.dram_tensor("btab", [128, T], F32, kind="ExternalInput")
    pswap_d = nc.dram_tensor("pswap", [128, 128], F32, kind="ExternalInput")
    trib_d = nc.dram_tensor("trib", [128, 128], BF16, kind="ExternalInput")
    identb_d = nc.dram_tensor("identb", [128, 128], BF16, kind="ExternalInput")
    identr_d = nc.dram_tensor("identr", [64, 64], F32, kind="ExternalInput")
    ones65_d = nc.dram_tensor("ones65", [65, 64], F32, kind="ExternalInput")
    ones16_d = nc.dram_tensor("ones16", [128, 16], BF16, kind="ExternalInput")
    zeros_d = nc.dram_tensor("zeros64", [64, T], BF16, kind="ExternalInput")
    y_d = nc.dram_tensor("y", [T, C], F32, kind="ExternalOutput")

    xt_r = xt_d.rearrange("(ko p) t -> p ko t", p=128)
    wproj_r = wproj_d.rearrange("(ko p) f -> p ko f", p=128).bitcast(F32R)

    with TileContext(nc) as tc:
        from contextlib import ExitStack

        with ExitStack() as ctx:
            const = ctx.enter_context(tc.tile_pool(name="const", bufs=1))
            pers = ctx.enter_context(tc.tile_pool(name="pers", bufs=1))
            # --- constants ---
            wproj_sb = const.tile([128, KT, FPROJ], F32R)
            wo_sb = const.tile([128, 2, C], BF16)
            atab = const.tile([128, T], F32)
            btab = const.tile([128, T], F32)
            pswap = const.tile([128, 128], F32R)
            trib = const.tile([128, 128], BF16)
            identb = const.tile([128, 128], BF16)
            identr = const.tile([128, 64], F32R)
            ones65 = const.tile([65, 64], F32R)

            # --- persistent activations ---
            qr = [pers.tile([128, T], BF16, name=f"qr{i}") for i in range(2)]
            # k^T zero-padded to 128 contraction rows: kr0 = [k; 0] for even
            # heads, kr1 = [0; k] for odd heads -> S matmuls engage the full
            # PE array while the zero half kills the other head's q rows.
            kr0 = pers.tile([128, T], BF16)
            kr1 = pers.tile([128, T], BF16)
            vsb = pers.tile([128, T // 128, 65], BF16)  # v natural + ones col
            opk = pers.tile([128, 2, T], BF16)  # packed normalized O^T for Wo
            kvp = pers.tile([128, T], F32R)  # k^T rows 0:64, v^T rows 64:128

            xpool = ctx.enter_context(tc.tile_pool(name="xp", bufs=2))
            tmp = ctx.enter_context(tc.tile_pool(name="tmp", bufs=2))
            ppool = ctx.enter_context(tc.tile_pool(name="pp", bufs=4))
            o65pool = ctx.enter_context(tc.tile_pool(name="o65p", bufs=2))
            rcpool = ctx.enter_context(tc.tile_pool(name="rc", bufs=2))
            ypool = ctx.enter_context(tc.tile_pool(name="yp", bufs=2))
            # PSUM: pp(2) + s(2x2) + o(2) = 8 banks
            pp_ps = ctx.enter_context(
                tc.tile_pool(name="ppps", bufs=2, space="PSUM")
            )
            spool = ctx.enter_context(
                tc.tile_pool(name="sps", bufs=2, space="PSUM")
            )
            opool = ctx.enter_context(
                tc.tile_pool(name="ops", bufs=2, space="PSUM")
            )

            xrows = {}

            def emit_x_dma(n):
                xr = xpool.tile([128, KT, 512], F32R, tag="x", name=f"x{n}")
                xrows[n] = xr
                if n == 0:
                    return  # row 0 loads per-k, interleaved with wproj
                for half in range(2):
                    ks = slice(4 * half, 4 * half + 4)
                    nc.sync.dma_start(
                        xr[:, ks], xt_r[:, ks, bass.ts(n, 512)].bitcast(F32R)
                    )

            def emit_preamble_dma():
                emit_x_dma(0)
                xr = xrows[0]
                for k in range(KT):
                    nc.sync.dma_start(wproj_sb[:, k], wproj_r[:, k])
                    nc.sync.dma_start(
                        xr[:, k], xt_r[:, k, bass.ts(0, 512)].bitcast(F32R)
                    )
                nc.sync.dma_start(atab[:], atab_d[:])
                nc.sync.dma_start(btab[:], btab_d[:])
                nc.sync.dma_start(pswap[:], pswap_d[:].bitcast(F32R))
                nc.sync.dma_start(identr[64:128, :], identr_d[:].bitcast(F32R))
                nc.sync.dma_start(kr0[64:128, :], zeros_d[:])
                nc.sync.dma_start(kr1[0:64, :], zeros_d[:])
                nc.sync.dma_start(vsb[:, :, 64], ones16_d[:])
                nc.sync.dma_start(trib[:], trib_d[:])
                nc.sync.dma_start(identb[:], identb_d[:])
                nc.sync.dma_start(ones65[:], ones65_d[:].bitcast(F32R))
                nc.sync.dma_start(
                    wo_sb[:], wo_d.rearrange("(ko p) c -> p ko c", p=128)
                )

            def emit_proj(n):
                """projections + RoPE for token block n -> qr/kr/vsb cols."""
                xr = xrows[n]
                for m in range(MT):
                    ps = pp_ps.tile([128, 512], F32, tag="pp", name=f"ps{m}")
                    for k in range(KT):
                        nc.tensor.matmul(
                            ps[:],
                            wproj_sb[:, k, bass.ts(m, 128)],
                            xr[:, k],
                            start=(k == 0),
                            stop=(k == KT - 1),
                        )
                    rows = 128 if m < 2 else 64
                    if m == 2:
                        plain = kvp[:, bass.ts(n, 512)]
                    else:
                        qt_t = tmp.tile([128, 512], F32R, tag="qt", name="qt")
                        plain = qt_t[:]
                    nc.vector.tensor_copy(plain, ps[:])
                    qsw = pp_ps.tile([128, 512], F32, tag="pp", name=f"qsw{m}")
                    nc.tensor.matmul(
                        qsw[0:rows],
                        pswap[0:rows, 0:rows],
                        plain[0:rows],
                        start=True,
                        stop=True,
                    )
                    t1 = tmp.tile([128, 512], F32, tag="t1")
                    nc.vector.tensor_tensor(
                        t1[0:rows],
                        plain[0:rows].bitcast(F32),
                        atab[0:rows, bass.ts(n, 512)],
                        mybir.AluOpType.mult,
                    )
                    t2 = tmp.tile([128, 512], F32, tag="t2")
                    nc.vector.tensor_tensor(
                        t2[0:rows],
                        qsw[0:rows],
                        btab[0:rows, bass.ts(n, 512)],
                        mybir.AluOpType.mult,
                    )
                    dest = qr[m] if m < 2 else kr0
                    nc.gpsimd.tensor_tensor(
                        dest[0:rows, bass.ts(n, 512)],
                        t1[0:rows],
                        t2[0:rows],
                        mybir.AluOpType.add,
                    )
                    if m == 2:
                        # duplicate k^T into kr1 rows 64:128
                        nc.vector.tensor_copy(
                            kr1[64:128, bass.ts(n, 512)],
                            kr0[0:64, bass.ts(n, 512)],
                        )
                        # v^T -> v natural (PE transpose per 128-token block)
                        for tt in range(4 * n, 4 * n + 4):
                            vt_ps = pp_ps.tile(
                                [128, 64], F32, tag="pp", name="vt"
                            )
                            nc.tensor.transpose(
                                vt_ps[:],
                                kvp[64:128, bass.ts(tt, 128)].bitcast(F32),
                                identr[64:128, :].bitcast(F32),
                            )
                            nc.vector.tensor_copy(vsb[:, tt, 0:64], vt_ps[:])

            def emit_attn_pair(n, hp, o65b):
                """S/exp/O for one head pair of query row n."""
                jq = n
                nkb = 4 * (jq + 1)
                qtile = qr[hp]
                heads = (2 * hp, 2 * hp + 1)
                o_ps = {
                    h: opool.tile([65, 512], F32, tag="o", name=f"o{h}")
                    for h in heads
                }
                pend = None
                for ksb in range(nkb // 2):
                    regions = []
                    for jk in range(2):
                        kb = 2 * ksb + jk
                        j = kb - 4 * jq
                        col0 = max(j, 0) * 128
                        regions.append((jk, col0, kb))
                    s_ps = {}
                    p_sb = {}
                    for h in heads:
                        s_ps[h] = spool.tile(
                            [128, 1024], F32, tag="s", name=f"s{h}"
                        )
                        p_sb[h] = ppool.tile(
                            [128, 1024], BF16, tag="p", name=f"pb{h}"
                        )
                    for jk, col0, kb in regions:
                        for h in heads:
                            krt = kr0 if h % 2 == 0 else kr1
                            nc.tensor.matmul(
                                s_ps[h][:, jk * 512 + col0 : jk * 512 + 512],
                                krt[:, bass.ts(kb, 128)],
                                qtile[:, jq * 512 + col0 : jq * 512 + 512],
                                start=True,
                                stop=(kb < 4 * jq),
                            )
                    for h in heads:
                        for jk, col0, kb in regions:
                            if kb - 4 * jq >= 0:
                                nc.tensor.matmul(
                                    s_ps[h][
                                        :,
                                        jk * 512 + col0 : jk * 512 + col0 + 128,
                                    ],
                                    identb[:],
                                    trib[:],
                                    start=False,
                                    stop=True,
                                )
                    for h in heads:
                        if regions[0][1] == 0 and regions[1][1] == 0:
                            nc.scalar.activation(
                                p_sb[h][:],
                                s_ps[h][:],
                                mybir.ActivationFunctionType.Exp,
                                scale=0.125,
                            )
                        else:
                            for jk, col0, kb in regions:
                                nc.scalar.activation(
                                    p_sb[h][:, jk * 512 + col0 : jk * 512 + 512],
                                    s_ps[h][:, jk * 512 + col0 : jk * 512 + 512],
                                    mybir.ActivationFunctionType.Exp,
                                    scale=0.125,
                                )
                    # O matmuls run one K-step behind S so the PE queue
                    # never head-of-line blocks on the scalar exps.
                    if pend is not None:
                        for h in heads:
                            for jk, col0, kb in pend[0]:
                                nc.tensor.matmul(
                                    o_ps[h][:, col0:512],
                                    vsb[:, kb, :],
                                    pend[1][h][
                                        :, jk * 512 + col0 : jk * 512 + 512
                                    ],
                                    start=(kb == 0),
                                    stop=(kb == nkb - 1),
                                )
                    pend = (regions, p_sb)
                for h in heads:
                    for jk, col0, kb in pend[0]:
                        nc.tensor.matmul(
                            o_ps[h][:, col0:512],
                            vsb[:, kb, :],
                            pend[1][h][:, jk * 512 + col0 : jk * 512 + 512],
                            start=(kb == 0),
                            stop=(kb == nkb - 1),
                        )
                for h in heads:
                    nc.vector.tensor_copy(o65b[:, h, :], o_ps[h][:])
                # batched 1/rowsum for the pair: Ln + Exp(-x) on the scalar
                # engine (both live in the natural_log_exp act table).
                lnd = rcpool.tile(
                    [1, 2, 512], F32, tag="lnd", name=f"ln{jq}_{hp}", bufs=2
                )
                nc.scalar.activation(
                    lnd[0:1, :, :],
                    o65b[64:65, 2 * hp : 2 * hp + 2, :].bitcast(F32),
                    mybir.ActivationFunctionType.Ln,
                )
                rc = rcpool.tile(
                    [1, 2, 512], F32R, tag="rc", name=f"rc{jq}_{hp}", bufs=2
                )
                nc.scalar.activation(
                    rc[0:1, :, :],
                    lnd[0:1, :, :],
                    mybir.ActivationFunctionType.Exp,
                    scale=-1.0,
                )
                return rc

            def emit_norm_pair(n, hp, o65b, rc):
                """broadcast 1/rowsum via PE, apply via DVE -> opk (bf16)."""
                jq = n
                for hh in range(2):
                    h = 2 * hp + hh
                    bc_ps = pp_ps.tile([64, 512], F32, tag="pp", name=f"bc{h}")
                    nc.tensor.matmul(
                        bc_ps[:],
                        ones65[0:1, :],
                        rc[0:1, hh, :],
                        start=True,
                        stop=True,
                    )
                    nc.vector.tensor_tensor(
                        opk[(h % 2) * 64 : (h % 2) * 64 + 64, h // 2, bass.ts(jq, 512)],
                        o65b[0:64, h, :].bitcast(F32),
                        bc_ps[:],
                        mybir.AluOpType.mult,
                    )

            def emit_wo(n):
                """Wo projection + writeout for the row's 4 token blocks."""
                for t in range(4 * n, 4 * n + 4):
                    y_sb = ypool.tile([128, C], F32, tag="y", name="ysb")
                    for nn in range(2):
                        wps = pp_ps.tile([128, 512], F32, tag="pp", name="wps")
                        for k in range(2):
                            nc.tensor.matmul(
                                wps[:],
                                opk[:, k, bass.ts(t, 128)],
                                wo_sb[:, k, bass.ts(nn, 512)],
                                start=(k == 0),
                                stop=(k == 1),
                            )
                        if (t + nn) % 2 == 0:
                            nc.vector.tensor_copy(
                                y_sb[:, bass.ts(nn, 512)], wps[:]
                            )
                        else:
                            nc.scalar.copy(y_sb[:, bass.ts(nn, 512)], wps[:])
                    nc.sync.dma_start(y_d[bass.ts(t, 128), :], y_sb[:])

            emit_preamble_dma()
            emit_proj(0)
            for n in range(NT):
                o65b = o65pool.tile(
                    [65, 4, 512], F32R, tag="o65", name=f"o65_{n}"
                )
                rc0 = emit_attn_pair(n, 0, o65b)
                rc1 = emit_attn_pair(n, 1, o65b)
                # hp0's broadcast runs while hp1's Ln/Exp is still going;
                # proj(n+1) then fills the PE during hp1's normalization.
                emit_norm_pair(n, 0, o65b, rc0)
                if n + 1 < NT:
                    emit_x_dma(n + 1)
                    emit_proj(n + 1)
                emit_norm_pair(n, 1, o65b, rc1)
                emit_wo(n)

    _split_excess_waits(nc)
    return nc


_NC_CACHE = None


def _get_nc():
    global _NC_CACHE
    if _NC_CACHE is None:
        _NC_CACHE = _build()
    return _NC_CACHE


def _host_prep(x, cos, sin, Wq, Wk, Wv, Wo):
    import ml_dtypes

    cos2 = np.asarray(cos, np.float32).reshape(T, HALF)  # [T, 32]
    sin2 = np.asarray(sin, np.float32).reshape(T, HALF)
    atab = np.tile(cos2.T, (4, 1))  # [128, T]
    btab = np.tile(np.vstack([sin2.T, -sin2.T]), (2, 1))  # [128, T]
    idx = np.arange(128)
    pswap = np.zeros((128, 128), np.float32)
    pswap[idx ^ 32, idx] = 1.0
    k_i = np.arange(128)[:, None]
    q_i = np.arange(128)[None, :]
    trib = np.where(k_i > q_i, np.float32(NEG), np.float32(0.0)).astype(
        ml_dtypes.bfloat16
    )
    identb = np.eye(128, dtype=ml_dtypes.bfloat16)
    identr = np.eye(64, dtype=np.float32)
    ones65 = np.ones((65, 64), np.float32)
    ones16 = np.ones((128, 16), ml_dtypes.bfloat16)
    zeros64 = np.zeros((64, T), ml_dtypes.bfloat16)

    in_maps = []
    for core in range(8):
        b, g = core // 4, core % 4
        xt = np.ascontiguousarray(np.asarray(x[b], np.float32).T)  # [C, T]
        wproj = np.ascontiguousarray(
            np.concatenate(
                [
                    Wq[:, g * FQ : (g + 1) * FQ],
                    Wk[:, g * D : (g + 1) * D],
                    Wv[:, g * D : (g + 1) * D],
                ],
                axis=1,
            ).astype(np.float32)
        )
        wo = np.ascontiguousarray(
            Wo[g * FQ : (g + 1) * FQ, :].astype(ml_dtypes.bfloat16)
        )
        in_maps.append(
            {
                "xt": xt,
                "wproj": wproj,
                "wo": wo,
                "atab": atab,
                "btab": btab,
                "pswap": pswap,
                "trib": trib,
                "identb": identb,
                "identr": identr,
                "ones65": ones65,
                "ones16": ones16,
                "zeros64": zeros64,
            }
        )
    return in_maps


def kernel(x, cos, sin, Wq, Wk, Wv, Wo, _want_trace=False, _trace_kwargs=None):
    nc = _get_nc()
    in_maps = _host_prep(x, cos, sin, Wq, Wk, Wv, Wo)
    kw = {}
    if _want_trace:
        kw = dict(trace=True, **(_trace_kwargs or {}))
    res = run_bass_kernel_spmd(nc, in_maps, list(range(8)), **kw)
    y = np.zeros((B, T, C), np.float32)
    for core in range(8):
        b = core // 4
        y[b] += res.results[core]["y"]
    if _want_trace:
        kernel.last_result = res
    return y


# revision 11
# speedup vs baseline: 1.6060x; 1.0335x over previous
"""Causal self-attention (GQA + RoPE) Trainium2 Bass kernel.

Sharding: 8 cores = batch(2) x kv-group(4). Each core computes its batch's
4 q-heads / 1 kv-head and a row-shard of the Wo projection; the 4 partial
outputs per batch are summed on host (all-reduce replacement).

Fused single-pass pipeline over 512-query rows: for each row n we
project+RoPE x block n, immediately run the causal attention row jq=n
(which only needs k/v blocks 0..4n+3, all available), normalize, and run
the Wo projection + y writeout for the row's 4 token blocks. This keeps
the PE dense (DVFS p-state ramps up), starts the softmax exps early, and
overlaps all DMA with compute.

q/k (post-RoPE), P (softmax probs), V, opk and Wo are bf16: same PE
cycles/row but no fp32r short-stream penalty, half the LDWEIGHTS cost and
SBUF traffic. S logits / rowsums / y accumulate in fp32.

Self-contained: hardcodes all shapes from the problem spec.
"""

import numpy as np

import concourse.bass as bass
import concourse.mybir as mybir
from concourse.tile import TileContext
from concourse.bass_utils import run_bass_kernel_spmd

F32 = mybir.dt.float32
F32R = mybir.dt.float32r
BF16 = mybir.dt.bfloat16

B, T, C = 2, 2048, 1024
H, HKV, D = 16, 4, 64
HALF = D // 2  # 32
GQ = H // HKV  # 4 q heads per group
FQ = GQ * D    # 256 q features per group
FPROJ = FQ + 2 * D  # 384: q(256) + k(64) + v(64)
NT = T // 512  # 4 row blocks of 512
KT = C // 128  # 8 contraction tiles
MT = FPROJ // 128  # 3 output row tiles (q01, q23, kv)
NEG = -1.0e9


def _split_excess_waits(nc, max_waits=1):
    """walrus here encodes at most one sync-wait per instruction; hoist the
    rest into standalone EventSemaphore instructions (raw-bass encoding)."""
    n = 0
    for fn in nc.m.functions:
        for bb in fn.blocks:
            new = []
            changed = False
            for inst in bb.instructions:
                si = inst.sync_info
                if si is not None and len(si.on_wait) > max_waits:
                    waits = list(si.on_wait)
                    for j, w in enumerate(waits[max_waits:]):
                        ev = mybir.InstEventSemaphore(
                            name=f"{inst.name}-ws{j}",
                            engine=inst.engine,
                            ins=[],
                            outs=[],
                            sync_info=mybir.SyncInfo(on_wait=[w], on_update=[]),
                        )
                        new.append(ev)
                        n += 1
                    inst.sync_info = mybir.SyncInfo(
                        on_wait=waits[:max_waits], on_update=list(si.on_update)
                    )
                    changed = True
                new.append(inst)
            if changed:
                bb.instructions = new
    return n


def _build():
    nc = bass.Bass()
    xt_d = nc.dram_tensor("xt", [C, T], F32, kind="ExternalInput")
    wproj_d = nc.dram_tensor("wproj", [C, FPROJ], F32, kind="ExternalInput")
    wo_d = nc.dram_tensor("wo", [FQ, C], BF16, kind="ExternalInput")
    atab_d = nc.dram_tensor("atab", [128, T], F32, kind="ExternalInput")
    btab_d = nc.dram_tensor("btab", [128, T], F32, kind="ExternalInput")
    pswap_d = nc.dram_tensor("pswap", [128, 128], F32, kind="ExternalInput")
    trib_d = nc.dram_tensor("trib", [128, 128], BF16, kind="ExternalInput")
    identb_d = nc.dram_tensor("identb", [128, 128], BF16, kind="ExternalInput")
    identr_d = nc.dram_tensor("identr", [64, 64], F32, kind="ExternalInput")
    ones65_d = nc.dram_tensor("ones65", [65, 64], F32, kind="ExternalInput")
    ones16_d = nc.dram_tensor("ones16", [128, 16], BF16, kind="ExternalInput")
    zeros_d = nc.dram_tensor("zeros64", [64, T], BF16, kind="ExternalInput")
    y_d = nc.dram_tensor("y", [T, C], F32, kind="ExternalOutput")

    xt_r = xt_d.rearrange("(ko p) t -> p ko t", p=128)
    wproj_r = wproj_d.rearrange("(ko p) f -> p ko f", p=128).bitcast(F32R)

    with TileContext(nc) as tc:
        from contextlib import ExitStack

        with ExitStack() as ctx:
            const = ctx.enter_context(tc.tile_pool(name="const", bufs=1))
            pers = ctx.enter_context(tc.tile_pool(name="pers", bufs=1))
            # --- constants ---
            wproj_sb = const.tile([128, KT, FPROJ], F32R)
            wo_sb = const.tile([128, 2, C], BF16)
            atab = const.tile([128, T], F32)
            btab = const.tile([128, T], F32)
            pswap = const.tile([128, 128], F32R)
            trib = const.tile([128, 128], BF16)
            identb = const.tile([128, 128], BF16)
            identr = const.tile([128, 64], F32R)
            ones65 = const.tile([65, 64], F32R)

            # --- persistent activations ---
            qr = [pers.tile([128, T], BF16, name=f"qr{i}") for i in range(2)]
            # k^T zero-padded to 128 contraction rows: kr0 = [k; 0] for even
            # heads, kr1 = [0; k] for odd heads -> S matmuls engage the full
            # PE array while the zero half kills the other head's q rows.
            kr0 = pers.tile([128, T], BF16)
            kr1 = pers.tile([128, T], BF16)
            vsb = pers.tile([128, T // 128, 65], BF16)  # v natural + ones col
            opk = pers.tile([128, 2, T], BF16)  # packed normalized O^T for Wo
            kvp = pers.tile([128, T], F32R)  # k^T rows 0:64, v^T rows 64:128

            xpool = ctx.enter_context(tc.tile_pool(name="xp", bufs=2))
            tmp = ctx.enter_context(tc.tile_pool(name="tmp", bufs=2))
            ppool = ctx.enter_context(tc.tile_pool(name="pp", bufs=4))
            o65pool = ctx.enter_context(tc.tile_pool(name="o65p", bufs=2))
            rcpool = ctx.enter_context(tc.tile_pool(name="rc", bufs=2))
            ypool = ctx.enter_context(tc.tile_pool(name="yp", bufs=2))
            # PSUM: pp(2) + s(2x2) + o(2) = 8 banks
            pp_ps = ctx.enter_context(
                tc.tile_pool(name="ppps", bufs=2, space="PSUM")
            )
            spool = ctx.enter_context(
                tc.tile_pool(name="sps", bufs=2, space="PSUM")
            )
            opool = ctx.enter_context(
                tc.tile_pool(name="ops", bufs=2, space="PSUM")
            )

            xrows = {}

            def emit_x_dma(n):
                xr = xpool.tile([128, KT, 512], F32R, tag="x", name=f"x{n}")
                xrows[n] = xr
                if n == 0:
                    return  # row 0 loads per-k, interleaved with wproj
                for half in range(2):
                    ks = slice(4 * half, 4 * half + 4)
                    nc.sync.dma_start(
                        xr[:, ks], xt_r[:, ks, bass.ts(n, 512)].bitcast(F32R)
                    )

            def emit_preamble_dma():
                emit_x_dma(0)
                xr = xrows[0]
                for k in range(KT):
                    nc.sync.dma_start(wproj_sb[:, k], wproj_r[:, k])
                    nc.sync.dma_start(
                        xr[:, k], xt_r[:, k, bass.ts(0, 512)].bitcast(F32R)
                    )
                nc.sync.dma_start(atab[:], atab_d[:])
                nc.sync.dma_start(btab[:], btab_d[:])
                nc.sync.dma_start(pswap[:], pswap_d[:].bitcast(F32R))
                nc.sync.dma_start(identr[64:128, :], identr_d[:].bitcast(F32R))
                nc.sync.dma_start(kr0[64:128, :], zeros_d[:])
                nc.sync.dma_start(kr1[0:64, :], zeros_d[:])
                nc.sync.dma_start(vsb[:, :, 64], ones16_d[:])
                nc.sync.dma_start(trib[:], trib_d[:])
                nc.sync.dma_start(identb[:], identb_d[:])
                nc.sync.dma_start(ones65[:], ones65_d[:].bitcast(F32R))
                nc.sync.dma_start(
                    wo_sb[:], wo_d.rearrange("(ko p) c -> p ko c", p=128)
                )

            def emit_proj(n):
                """projections + RoPE for token block n -> qr/kr/vsb cols."""
                xr = xrows[n]
                for m in range(MT):
                    ps = pp_ps.tile([128, 512], F32, tag="pp", name=f"ps{m}")
                    for k in range(KT):
                        nc.tensor.matmul(
                            ps[:],
                            wproj_sb[:, k, bass.ts(m, 128)],
                            xr[:, k],
                            start=(k == 0),
                            stop=(k == KT - 1),
                        )
                    rows = 128 if m < 2 else 64
                    if m == 2:
                        plain = kvp[:, bass.ts(n, 512)]
                    else:
                        qt_t = tmp.tile([128, 512], F32R, tag="qt", name="qt")
                        plain = qt_t[:]
                    nc.vector.tensor_copy(plain, ps[:])
                    qsw = pp_ps.tile([128, 512], F32, tag="pp", name=f"qsw{m}")
                    nc.tensor.matmul(
                        qsw[0:rows],
                        pswap[0:rows, 0:rows],
                        plain[0:rows],
                        start=True,
                        stop=True,
                    )
                    t1 = tmp.tile([128, 512], F32, tag="t1")
                    nc.vector.tensor_tensor(
                        t1[0:rows],
                        plain[0:rows].bitcast(F32),
                        atab[0:rows, bass.ts(n, 512)],
                        mybir.AluOpType.mult,
                    )
                    t2 = tmp.tile([128, 512], F32, tag="t2")
                    nc.vector.tensor_tensor(
                        t2[0:rows],
                        qsw[0:rows],
                        btab[0:rows, bass.ts(n, 512)],
                        mybir.AluOpType.mult,
                    )
                    dest = qr[m] if m < 2 else kr0
                    nc.gpsimd.tensor_tensor(
                        dest[0:rows, bass.ts(n, 512)],
                        t1[0:rows],
                        t2[0:rows],
                        mybir.AluOpType.add,
                    )
                    if m == 2:
                        # duplicate k^T into kr1 rows 64:128
                        nc.vector.tensor_copy(
                            kr1[64:128, bass.ts(n, 512)],
                            kr0[0:64, bass.ts(n, 512)],
                        )
                        # v^T -> v natural (PE transpose per 128-token block)
                        for tt in range(4 * n, 4 * n + 4):
                            vt_ps = pp_ps.tile(
                                [128, 64], F32, tag="pp", name="vt"
                            )
                            nc.tensor.transpose(
                                vt_ps[:],
                                kvp[64:128, bass.ts(tt, 128)].bitcast(F32),
                                identr[64:128, :].bitcast(F32),
                            )
                            nc.vector.tensor_copy(vsb[:, tt, 0:64], vt_ps[:])

            def emit_attn_pair(n, hp, o65b):
                """S/exp/O for one head pair of query row n."""
                jq = n
                nkb = 4 * (jq + 1)
                qtile = qr[hp]
                heads = (2 * hp, 2 * hp + 1)
                o_ps = {
                    h: opool.tile([65, 512], F32, tag="o", name=f"o{h}")
                    for h in heads
                }
                pend = None
                for ksb in range(nkb // 2):
                    regions = []
                    for jk in range(2):
                        kb = 2 * ksb + jk
                        j = kb - 4 * jq
                        col0 = max(j, 0) * 128
                        regions.append((jk, col0, kb))
                    s_ps = {}
                    p_sb = {}
                    for h in heads:
                        s_ps[h] = spool.tile(
                            [128, 1024], F32, tag="s", name=f"s{h}"
                        )
                        p_sb[h] = ppool.tile(
                            [128, 1024], BF16, tag="p", name=f"pb{h}"
                        )
                    for jk, col0, kb in regions:
                        for h in heads:
                            krt = kr0 if h % 2 == 0 else kr1
                            nc.tensor.matmul(
                                s_ps[h][:, jk * 512 + col0 : jk * 512 + 512],
                                krt[:, bass.ts(kb, 128)],
                                qtile[:, jq * 512 + col0 : jq * 512 + 512],
                                start=True,
                                stop=True,
                            )
                    for h in heads:
                        if regions[0][1] == 0 and regions[1][1] == 0:
                            nc.scalar.activation(
                                p_sb[h][:],
                                s_ps[h][:],
                                mybir.ActivationFunctionType.Exp,
                                scale=0.125,
                            )
                        else:
                            for jk, col0, kb in regions:
                                nc.scalar.activation(
                                    p_sb[h][:, jk * 512 + col0 : jk * 512 + 512],
                                    s_ps[h][:, jk * 512 + col0 : jk * 512 + 512],
                                    mybir.ActivationFunctionType.Exp,
                                    scale=0.125,
                                )
                    for h in heads:
                        for jk, col0, kb in regions:
                            if kb - 4 * jq >= 0:
                                nc.gpsimd.tensor_tensor(
                                    p_sb[h][
                                        :,
                                        jk * 512 + col0 : jk * 512 + col0 + 128,
                                    ],
                                    p_sb[h][
                                        :,
                                        jk * 512 + col0 : jk * 512 + col0 + 128,
                                    ],
                                    trib[:],
                                    mybir.AluOpType.mult,
                                )
                    # O matmuls run one K-step behind S so the PE queue
                    # never head-of-line blocks on the scalar exps.
                    if pend is not None:
                        for h in heads:
                            for jk, col0, kb in pend[0]:
                                nc.tensor.matmul(
                                    o_ps[h][:, col0:512],
                                    vsb[:, kb, :],
                                    pend[1][h][
                                        :, jk * 512 + col0 : jk * 512 + 512
                                    ],
                                    start=(kb == 0),
                                    stop=(kb == nkb - 1),
                                )
                    pend = (regions, p_sb)
                for h in heads:
                    for jk, col0, kb in pend[0]:
                        nc.tensor.matmul(
                            o_ps[h][:, col0:512],
                            vsb[:, kb, :],
                            pend[1][h][:, jk * 512 + col0 : jk * 512 + 512],
                            start=(kb == 0),
                            stop=(kb == nkb - 1),
                        )
                for h in heads:
                    nc.vector.tensor_copy(o65b[:, h, :], o_ps[h][:])
                # batched 1/rowsum for the pair: Ln + Exp(-x) on the scalar
                # engine (both live in the natural_log_exp act table).
                lnd = rcpool.tile(
                    [1, 2, 512], F32, tag="lnd", name=f"ln{jq}_{hp}", bufs=2
                )
                nc.scalar.activation(
                    lnd[0:1, :, :],
                    o65b[64:65, 2 * hp : 2 * hp + 2, :].bitcast(F32),
                    mybir.ActivationFunctionType.Ln,
                )
                rc = rcpool.tile(
                    [1, 2, 512], F32R, tag="rc", name=f"rc{jq}_{hp}", bufs=2
                )
                nc.scalar.activation(
                    rc[0:1, :, :],
                    lnd[0:1, :, :],
                    mybir.ActivationFunctionType.Exp,
                    scale=-1.0,
                )
                return rc

            def emit_norm_pair(n, hp, o65b, rc):
                """broadcast 1/rowsum via PE, apply via DVE -> opk (bf16)."""
                jq = n
                for hh in range(2):
                    h = 2 * hp + hh
                    bc_ps = pp_ps.tile([64, 512], F32, tag="pp", name=f"bc{h}")
                    nc.tensor.matmul(
                        bc_ps[:],
                        ones65[0:1, :],
                        rc[0:1, hh, :],
                        start=True,
                        stop=True,
                    )
                    nc.vector.tensor_tensor(
                        opk[(h % 2) * 64 : (h % 2) * 64 + 64, h // 2, bass.ts(jq, 512)],
                        o65b[0:64, h, :].bitcast(F32),
                        bc_ps[:],
                        mybir.AluOpType.mult,
                    )

            def emit_wo(n):
                """Wo projection + writeout for the row's 4 token blocks."""
                for t in range(4 * n, 4 * n + 4):
                    y_sb = ypool.tile([128, C], F32, tag="y", name="ysb")
                    for nn in range(2):
                        wps = pp_ps.tile([128, 512], F32, tag="pp", name="wps")
                        for k in range(2):
                            nc.tensor.matmul(
                                wps[:],
                                opk[:, k, bass.ts(t, 128)],
                                wo_sb[:, k, bass.ts(nn, 512)],
                                start=(k == 0),
                                stop=(k == 1),
                            )
                        nc.vector.tensor_copy(y_sb[:, bass.ts(nn, 512)], wps[:])
                    nc.sync.dma_start(y_d[bass.ts(t, 128), :], y_sb[:])

            emit_preamble_dma()
            emit_proj(0)
            for n in range(NT):
                o65b = o65pool.tile(
                    [65, 4, 512], F32R, tag="o65", name=f"o65_{n}"
                )
                rc0 = emit_attn_pair(n, 0, o65b)
                rc1 = emit_attn_pair(n, 1, o65b)
                # proj(n+1) fills the PE while the Ln/Exp chains run on the
                # scalar engine; the broadcasts follow.
                if n + 1 < NT:
                    emit_x_dma(n + 1)
                    emit_proj(n + 1)
                emit_norm_pair(n, 0, o65b, rc0)
                emit_norm_pair(n, 1, o65b, rc1)
                emit_wo(n)

    _split_excess_waits(nc)
    return nc


_NC_CACHE = None


def _get_nc():
    global _NC_CACHE
    if _NC_CACHE is None:
        _NC_CACHE = _build()
    return _NC_CACHE


def _host_prep(x, cos, sin, Wq, Wk, Wv, Wo):
    import ml_dtypes

    cos2 = np.asarray(cos, np.float32).reshape(T, HALF)  # [T, 32]
    sin2 = np.asarray(sin, np.float32).reshape(T, HALF)
    atab = np.tile(cos2.T, (4, 1))  # [128, T]
    btab = np.tile(np.vstack([sin2.T, -sin2.T]), (2, 1))  # [128, T]
    idx = np.arange(128)
    pswap = np.zeros((128, 128), np.float32)
    pswap[idx ^ 32, idx] = 1.0
    k_i = np.arange(128)[:, None]
    q_i = np.arange(128)[None, :]
    trib = np.where(k_i > q_i, np.float32(0.0), np.float32(1.0)).astype(
        ml_dtypes.bfloat16
    )
    identb = np.eye(128, dtype=ml_dtypes.bfloat16)
    identr = np.eye(64, dtype=np.float32)
    ones65 = np.ones((65, 64), np.float32)
    ones16 = np.ones((128, 16), ml_dtypes.bfloat16)
    zeros64 = np.zeros((64, T), ml_dtypes.bfloat16)

    in_maps = []
    for core in range(8):
        b, g = core // 4, core % 4
        xt = np.ascontiguousarray(np.asarray(x[b], np.float32).T)  # [C, T]
        wproj = np.ascontiguousarray(
            np.concatenate(
                [
                    Wq[:, g * FQ : (g + 1) * FQ],
                    Wk[:, g * D : (g + 1) * D],
                    Wv[:, g * D : (g + 1) * D],
                ],
                axis=1,
            ).astype(np.float32)
        )
        wo = np.ascontiguousarray(
            Wo[g * FQ : (g + 1) * FQ, :].astype(ml_dtypes.bfloat16)
        )
        in_maps.append(
            {
                "xt": xt,
                "wproj": wproj,
                "wo": wo,
                "atab": atab,
                "btab": btab,
                "pswap": pswap,
                "trib": trib,
                "identb": identb,
                "identr": identr,
                "ones65": ones65,
                "ones16": ones16,
                "zeros64": zeros64,
            }
        )
    return in_maps


def kernel(x, cos, sin, Wq, Wk, Wv, Wo, _want_trace=False, _trace_kwargs=None):
    nc = _get_nc()
    in_maps = _host_prep(x, cos, sin, Wq, Wk, Wv, Wo)
    kw = {}
    if _want_trace:
        kw = dict(trace=True, **(_trace_kwargs or {}))
    res = run_bass_kernel_spmd(nc, in_maps, list(range(8)), **kw)
    y = np.zeros((B, T, C), np.float32)
    for core in range(8):
        b = core // 4
        y[b] += res.results[core]["y"]
    if _want_trace:
        kernel.last_result = res
    return y


# revision 13
# speedup vs baseline: 1.7391x; 1.0829x over previous
"""Causal self-attention (GQA + RoPE) Trainium2 Bass kernel.

Sharding: 8 cores = batch(2) x kv-group(4). Each core computes its batch's
4 q-heads / 1 kv-head and a row-shard of the Wo projection; the 4 partial
outputs per batch are summed on host (all-reduce replacement).

Fused single-pass pipeline over 512-query rows: for each row n we
project+RoPE x block n, immediately run the causal attention row jq=n
(which only needs k/v blocks 0..4n+3, all available), normalize, and run
the Wo projection + y writeout for the row's 4 token blocks. This keeps
the PE dense (DVFS p-state ramps up), starts the softmax exps early, and
overlaps all DMA with compute.

q/k (post-RoPE), P (softmax probs), V, opk and Wo are bf16: same PE
cycles/row but no fp32r short-stream penalty, half the LDWEIGHTS cost and
SBUF traffic. S logits / rowsums / y accumulate in fp32.

Self-contained: hardcodes all shapes from the problem spec.
"""

import numpy as np

import concourse.bass as bass
import concourse.mybir as mybir
from concourse.tile import TileContext
from concourse.bass_utils import run_bass_kernel_spmd

F32 = mybir.dt.float32
F32R = mybir.dt.float32r
BF16 = mybir.dt.bfloat16

B, T, C = 2, 2048, 1024
H, HKV, D = 16, 4, 64
HALF = D // 2  # 32
GQ = H // HKV  # 4 q heads per group
FQ = GQ * D    # 256 q features per group
FPROJ = FQ + 2 * D  # 384: q(256) + k(64) + v(64)
NT = T // 512  # 4 row blocks of 512
KT = C // 128  # 8 contraction tiles
MT = FPROJ // 128  # 3 output row tiles (q01, q23, kv)
NEG = -1.0e9


def _split_excess_waits(nc, max_waits=1):
    """walrus here encodes at most one sync-wait per instruction; hoist the
    rest into standalone EventSemaphore instructions (raw-bass encoding)."""
    n = 0
    for fn in nc.m.functions:
        for bb in fn.blocks:
            new = []
            changed = False
            for inst in bb.instructions:
                si = inst.sync_info
                if si is not None and len(si.on_wait) > max_waits:
                    waits = list(si.on_wait)
                    for j, w in enumerate(waits[max_waits:]):
                        ev = mybir.InstEventSemaphore(
                            name=f"{inst.name}-ws{j}",
                            engine=inst.engine,
                            ins=[],
                            outs=[],
                            sync_info=mybir.SyncInfo(on_wait=[w], on_update=[]),
                        )
                        new.append(ev)
                        n += 1
                    inst.sync_info = mybir.SyncInfo(
                        on_wait=waits[:max_waits], on_update=list(si.on_update)
                    )
                    changed = True
                new.append(inst)
            if changed:
                bb.instructions = new
    return n


def _build():
    nc = bass.Bass()
    xt_d = nc.dram_tensor("xt", [C, T], BF16, kind="ExternalInput")
    wproj_d = nc.dram_tensor("wproj", [C, FPROJ], BF16, kind="ExternalInput")
    wo_d = nc.dram_tensor("wo", [FQ, C], BF16, kind="ExternalInput")
    atab_d = nc.dram_tensor("atab", [128, T], BF16, kind="ExternalInput")
    btab_d = nc.dram_tensor("btab", [128, T], BF16, kind="ExternalInput")
    pswap_d = nc.dram_tensor("pswap", [128, 128], BF16, kind="ExternalInput")
    trib_d = nc.dram_tensor("trib", [128, 128], BF16, kind="ExternalInput")
    identr_d = nc.dram_tensor("identr", [64, 64], BF16, kind="ExternalInput")
    ones65_d = nc.dram_tensor("ones65", [65, 64], F32, kind="ExternalInput")
    zeros_d = nc.dram_tensor("zeros64", [64, T], BF16, kind="ExternalInput")
    y_d = nc.dram_tensor("y", [T, C], F32, kind="ExternalOutput")

    xt_r = xt_d.rearrange("(ko p) t -> p ko t", p=128)
    wproj_r = wproj_d.rearrange("(ko p) f -> p ko f", p=128)

    with TileContext(nc) as tc:
        from contextlib import ExitStack

        with ExitStack() as ctx:
            const = ctx.enter_context(tc.tile_pool(name="const", bufs=1))
            pers = ctx.enter_context(tc.tile_pool(name="pers", bufs=1))
            # --- constants ---
            wproj_sb = const.tile([128, KT, FPROJ], BF16)
            wo_sb = const.tile([128, 2, C], BF16)
            atab = const.tile([128, T], BF16)
            btab = const.tile([128, T], BF16)
            pswap = const.tile([128, 128], BF16)
            trib = const.tile([128, 128], BF16)
            identr = const.tile([128, 64], BF16)
            ones65 = const.tile([65, 64], F32R)

            # --- persistent activations ---
            qr = [pers.tile([128, T], BF16, name=f"qr{i}") for i in range(2)]
            # k^T zero-padded to 128 contraction rows: kr0 = [k; 0] for even
            # heads, kr1 = [0; k] for odd heads -> S matmuls engage the full
            # PE array while the zero half kills the other head's q rows.
            kr0 = pers.tile([128, T], BF16)
            kr1 = pers.tile([128, T], BF16)
            vsb = pers.tile([128, T // 128, 65], BF16)  # v natural + ones col
            opk = pers.tile([128, 2, T], BF16)  # packed normalized O^T for Wo
            kvp = pers.tile([128, T], BF16)  # k^T rows 0:64, v^T rows 64:128

            xpool = ctx.enter_context(tc.tile_pool(name="xp", bufs=2))
            tmp = ctx.enter_context(tc.tile_pool(name="tmp", bufs=2))
            ppool = ctx.enter_context(tc.tile_pool(name="pp", bufs=4))
            o65pool = ctx.enter_context(tc.tile_pool(name="o65p", bufs=2))
            rcpool = ctx.enter_context(tc.tile_pool(name="rc", bufs=2))
            ypool = ctx.enter_context(tc.tile_pool(name="yp", bufs=2))
            # PSUM: pp(2) + s(2x2) + o(2) = 8 banks
            pp_ps = ctx.enter_context(
                tc.tile_pool(name="ppps", bufs=2, space="PSUM")
            )
            spool = ctx.enter_context(
                tc.tile_pool(name="sps", bufs=2, space="PSUM")
            )
            opool = ctx.enter_context(
                tc.tile_pool(name="ops", bufs=2, space="PSUM")
            )

            xrows = {}

            def emit_x_dma(n):
                xr = xpool.tile([128, KT, 512], BF16, tag="x", name=f"x{n}")
                xrows[n] = xr
                if n == 0:
                    return  # row 0 loads per-k, interleaved with wproj
                for half in range(2):
                    ks = slice(4 * half, 4 * half + 4)
                    nc.sync.dma_start(
                        xr[:, ks], xt_r[:, ks, bass.ts(n, 512)]
                    )

            def emit_preamble_dma():
                emit_x_dma(0)
                xr = xrows[0]
                nc.gpsimd.memset(vsb[:, :, 64], 1.0)
                for k in range(2):
                    nc.sync.dma_start(wproj_sb[:, k], wproj_r[:, k])
                    nc.sync.dma_start(xr[:, k], xt_r[:, k, bass.ts(0, 512)])
                nc.sync.dma_start(atab[:], atab_d[:])
                nc.sync.dma_start(btab[:], btab_d[:])
                nc.sync.dma_start(pswap[:], pswap_d[:])
                nc.sync.dma_start(identr[64:128, :], identr_d[:])
                for k in range(2, KT):
                    nc.sync.dma_start(wproj_sb[:, k], wproj_r[:, k])
                    nc.sync.dma_start(xr[:, k], xt_r[:, k, bass.ts(0, 512)])
                nc.sync.dma_start(kr0[64:128, :], zeros_d[:])
                nc.sync.dma_start(kr1[0:64, :], zeros_d[:])
                nc.sync.dma_start(trib[:], trib_d[:])
                nc.sync.dma_start(ones65[:], ones65_d[:].bitcast(F32R))
                nc.sync.dma_start(
                    wo_sb[:], wo_d.rearrange("(ko p) c -> p ko c", p=128)
                )

            def emit_proj(n):
                """projections + RoPE for token block n -> qr/kr/vsb cols."""
                xr = xrows[n]
                for m in range(MT):
                    ps = pp_ps.tile([128, 512], F32, tag="pp", name=f"ps{m}")
                    for k in range(KT):
                        nc.tensor.matmul(
                            ps[:],
                            wproj_sb[:, k, bass.ts(m, 128)],
                            xr[:, k],
                            start=(k == 0),
                            stop=(k == KT - 1),
                        )
                    rows = 128 if m < 2 else 64
                    if m == 2:
                        plain = kvp[:, bass.ts(n, 512)]
                    else:
                        qt_t = tmp.tile([128, 512], BF16, tag="qt", name="qt")
                        plain = qt_t[:]
                    nc.vector.tensor_copy(plain, ps[:])
                    qsw = pp_ps.tile([128, 512], F32, tag="pp", name=f"qsw{m}")
                    nc.tensor.matmul(
                        qsw[0:rows],
                        pswap[0:rows, 0:rows],
                        plain[0:rows],
                        start=True,
                        stop=True,
                    )
                    t1 = tmp.tile([128, 512], BF16, tag="t1")
                    nc.vector.tensor_tensor(
                        t1[0:rows],
                        plain[0:rows],
                        atab[0:rows, bass.ts(n, 512)],
                        mybir.AluOpType.mult,
                    )
                    t2 = tmp.tile([128, 512], BF16, tag="t2")
                    nc.vector.tensor_tensor(
                        t2[0:rows],
                        qsw[0:rows],
                        btab[0:rows, bass.ts(n, 512)],
                        mybir.AluOpType.mult,
                    )
                    dest = qr[m] if m < 2 else kr0
                    nc.gpsimd.tensor_tensor(
                        dest[0:rows, bass.ts(n, 512)],
                        t1[0:rows],
                        t2[0:rows],
                        mybir.AluOpType.add,
                    )
                    if m == 2:
                        # duplicate k^T into kr1 rows 64:128
                        nc.vector.tensor_copy(
                            kr1[64:128, bass.ts(n, 512)],
                            kr0[0:64, bass.ts(n, 512)],
                        )
                        # v^T -> v natural (PE transpose per 128-token block)
                        for tt in range(4 * n, 4 * n + 4):
                            vt_ps = pp_ps.tile(
                                [128, 64], BF16, tag="pp", name="vt"
                            )
                            nc.tensor.transpose(
                                vt_ps[:],
                                kvp[64:128, bass.ts(tt, 128)],
                                identr[64:128, :],
                            )
                            nc.vector.tensor_copy(vsb[:, tt, 0:64], vt_ps[:])

            def emit_attn_pair(n, hp, o65b):
                """S/exp/O for one head pair of query row n."""
                jq = n
                nkb = 4 * (jq + 1)
                qtile = qr[hp]
                heads = (2 * hp, 2 * hp + 1)
                o_ps = {
                    h: opool.tile([65, 512], F32, tag="o", name=f"o{h}")
                    for h in heads
                }
                pend = None
                for ksb in range(nkb // 2):
                    regions = []
                    for jk in range(2):
                        kb = 2 * ksb + jk
                        j = kb - 4 * jq
                        col0 = max(j, 0) * 128
                        regions.append((jk, col0, kb))
                    s_ps = {}
                    p_sb = {}
                    for h in heads:
                        s_ps[h] = spool.tile(
                            [128, 1024], F32, tag="s", name=f"s{h}"
                        )
                        p_sb[h] = ppool.tile(
                            [128, 1024], BF16, tag="p", name=f"pb{h}"
                        )
                    for jk, col0, kb in regions:
                        for h in heads:
                            krt = kr0 if h % 2 == 0 else kr1
                            nc.tensor.matmul(
                                s_ps[h][:, jk * 512 + col0 : jk * 512 + 512],
                                krt[:, bass.ts(kb, 128)],
                                qtile[:, jq * 512 + col0 : jq * 512 + 512],
                                start=True,
                                stop=True,
                            )
                    for h in heads:
                        if regions[0][1] == 0 and regions[1][1] == 0:
                            nc.scalar.activation(
                                p_sb[h][:],
                                s_ps[h][:],
                                mybir.ActivationFunctionType.Exp,
                                scale=0.125,
                            )
                        else:
                            for jk, col0, kb in regions:
                                nc.scalar.activation(
                                    p_sb[h][:, jk * 512 + col0 : jk * 512 + 512],
                                    s_ps[h][:, jk * 512 + col0 : jk * 512 + 512],
                                    mybir.ActivationFunctionType.Exp,
                                    scale=0.125,
                                )
                    for h in heads:
                        for jk, col0, kb in regions:
                            if kb - 4 * jq >= 0:
                                nc.gpsimd.tensor_tensor(
                                    p_sb[h][
                                        :,
                                        jk * 512 + col0 : jk * 512 + col0 + 128,
                                    ],
                                    p_sb[h][
                                        :,
                                        jk * 512 + col0 : jk * 512 + col0 + 128,
                                    ],
                                    trib[:],
                                    mybir.AluOpType.mult,
                                )
                    # O matmuls run one K-step behind S so the PE queue
                    # never head-of-line blocks on the scalar exps.
                    if pend is not None:
                        for h in heads:
                            for jk, col0, kb in pend[0]:
                                nc.tensor.matmul(
                                    o_ps[h][:, col0:512],
                                    vsb[:, kb, :],
                                    pend[1][h][
                                        :, jk * 512 + col0 : jk * 512 + 512
                                    ],
                                    start=(kb == 0),
                                    stop=(kb == nkb - 1),
                                )
                    pend = (regions, p_sb)
                for h in heads:
                    for jk, col0, kb in pend[0]:
                        nc.tensor.matmul(
                            o_ps[h][:, col0:512],
                            vsb[:, kb, :],
                            pend[1][h][:, jk * 512 + col0 : jk * 512 + 512],
                            start=(kb == 0),
                            stop=(kb == nkb - 1),
                        )
                for h in heads:
                    nc.vector.tensor_copy(o65b[:, h, :], o_ps[h][:])
                # batched 1/rowsum for the pair: Ln + Exp(-x) on the scalar
                # engine (both live in the natural_log_exp act table).
                lnd = rcpool.tile(
                    [1, 2, 512], F32, tag="lnd", name=f"ln{jq}_{hp}", bufs=2
                )
                nc.scalar.activation(
                    lnd[0:1, :, :],
                    o65b[64:65, 2 * hp : 2 * hp + 2, :].bitcast(F32),
                    mybir.ActivationFunctionType.Ln,
                )
                rc = rcpool.tile(
                    [1, 2, 512], F32R, tag="rc", name=f"rc{jq}_{hp}", bufs=2
                )
                nc.scalar.activation(
                    rc[0:1, :, :],
                    lnd[0:1, :, :],
                    mybir.ActivationFunctionType.Exp,
                    scale=-1.0,
                )
                return rc

            def emit_norm_pair(n, hp, o65b, rc):
                """broadcast 1/rowsum via PE, apply via DVE -> opk (bf16)."""
                jq = n
                for hh in range(2):
                    h = 2 * hp + hh
                    bc_ps = pp_ps.tile([64, 512], F32, tag="pp", name=f"bc{h}")
                    nc.tensor.matmul(
                        bc_ps[:],
                        ones65[0:1, :],
                        rc[0:1, hh, :],
                        start=True,
                        stop=True,
                    )
                    nc.vector.tensor_tensor(
                        opk[(h % 2) * 64 : (h % 2) * 64 + 64, h // 2, bass.ts(jq, 512)],
                        o65b[0:64, h, :].bitcast(F32),
                        bc_ps[:],
                        mybir.AluOpType.mult,
                    )

            def emit_wo(n):
                """Wo projection + writeout for the row's 4 token blocks."""
                for t in range(4 * n, 4 * n + 4):
                    y_sb = ypool.tile([128, C], F32, tag="y", name="ysb")
                    for nn in range(2):
                        wps = pp_ps.tile([128, 512], F32, tag="pp", name="wps")
                        for k in range(2):
                            nc.tensor.matmul(
                                wps[:],
                                opk[:, k, bass.ts(t, 128)],
                                wo_sb[:, k, bass.ts(nn, 512)],
                                start=(k == 0),
                                stop=(k == 1),
                            )
                        nc.vector.tensor_copy(y_sb[:, bass.ts(nn, 512)], wps[:])
                    nc.sync.dma_start(y_d[bass.ts(t, 128), :], y_sb[:])

            emit_preamble_dma()
            emit_proj(0)
            for n in range(NT):
                o65b = o65pool.tile(
                    [65, 4, 512], F32R, tag="o65", name=f"o65_{n}"
                )
                rc0 = emit_attn_pair(n, 0, o65b)
                rc1 = emit_attn_pair(n, 1, o65b)
                # proj(n+1) fills the PE while the Ln/Exp chains run on the
                # scalar engine; the broadcasts follow.
                if n + 1 < NT:
                    emit_x_dma(n + 1)
                    emit_proj(n + 1)
                emit_norm_pair(n, 0, o65b, rc0)
                emit_norm_pair(n, 1, o65b, rc1)
                emit_wo(n)

    _split_excess_waits(nc)
    return nc


_NC_CACHE = None


def _get_nc():
    global _NC_CACHE
    if _NC_CACHE is None:
        _NC_CACHE = _build()
    return _NC_CACHE


def _host_prep(x, cos, sin, Wq, Wk, Wv, Wo):
    import ml_dtypes

    cos2 = np.asarray(cos, np.float32).reshape(T, HALF)  # [T, 32]
    sin2 = np.asarray(sin, np.float32).reshape(T, HALF)
    atab = np.tile(cos2.T, (4, 1)).astype(ml_dtypes.bfloat16)  # [128, T]
    btab = np.tile(np.vstack([sin2.T, -sin2.T]), (2, 1)).astype(
        ml_dtypes.bfloat16
    )
    idx = np.arange(128)
    pswap = np.zeros((128, 128), ml_dtypes.bfloat16)
    pswap[idx ^ 32, idx] = 1.0
    k_i = np.arange(128)[:, None]
    q_i = np.arange(128)[None, :]
    trib = np.where(k_i > q_i, np.float32(0.0), np.float32(1.0)).astype(
        ml_dtypes.bfloat16
    )
    identr = np.eye(64, dtype=ml_dtypes.bfloat16)
    ones65 = np.ones((65, 64), np.float32)
    zeros64 = np.zeros((64, T), ml_dtypes.bfloat16)

    in_maps = []
    for core in range(8):
        b, g = core // 4, core % 4
        xt = np.ascontiguousarray(
            np.asarray(x[b], np.float32).T.astype(ml_dtypes.bfloat16)
        )  # [C, T]
        wproj = np.ascontiguousarray(
            np.concatenate(
                [
                    Wq[:, g * FQ : (g + 1) * FQ],
                    Wk[:, g * D : (g + 1) * D],
                    Wv[:, g * D : (g + 1) * D],
                ],
                axis=1,
            ).astype(ml_dtypes.bfloat16)
        )
        wo = np.ascontiguousarray(
            Wo[g * FQ : (g + 1) * FQ, :].astype(ml_dtypes.bfloat16)
        )
        in_maps.append(
            {
                "xt": xt,
                "wproj": wproj,
                "wo": wo,
                "atab": atab,
                "btab": btab,
                "pswap": pswap,
                "trib": trib,
                "identr": identr,
                "ones65": ones65,
                "zeros64": zeros64,
            }
        )
    return in_maps


def kernel(x, cos, sin, Wq, Wk, Wv, Wo, _want_trace=False, _trace_kwargs=None):
    nc = _get_nc()
    in_maps = _host_prep(x, cos, sin, Wq, Wk, Wv, Wo)
    kw = {}
    if _want_trace:
        kw = dict(trace=True, **(_trace_kwargs or {}))
    res = run_bass_kernel_spmd(nc, in_maps, list(range(8)), **kw)
    y = np.zeros((B, T, C), np.float32)
    for core in range(8):
        b = core // 4
        y[b] += res.results[core]["y"]
    if _want_trace:
        kernel.last_result = res
    return y


# revision 16
# speedup vs baseline: 1.7538x; 1.0084x over previous
"""Causal self-attention (GQA + RoPE) Trainium2 Bass kernel.

Sharding: 8 cores = batch(2) x kv-group(4). Each core computes its batch's
4 q-heads / 1 kv-head and a row-shard of the Wo projection; the 4 partial
outputs per batch are summed on host (all-reduce replacement).

Fused single-pass pipeline over 512-query rows: for each row n we
project+RoPE x block n, immediately run the causal attention row jq=n
(which only needs k/v blocks 0..4n+3, all available), normalize, and run
the Wo projection + y writeout for the row's 4 token blocks. This keeps
the PE dense (DVFS p-state ramps up), starts the softmax exps early, and
overlaps all DMA with compute.

q/k (post-RoPE), P (softmax probs), V, opk and Wo are bf16: same PE
cycles/row but no fp32r short-stream penalty, half the LDWEIGHTS cost and
SBUF traffic. S logits / rowsums / y accumulate in fp32.

Self-contained: hardcodes all shapes from the problem spec.
"""

import numpy as np

import concourse.bass as bass
import concourse.mybir as mybir
from concourse.tile import TileContext
from concourse.bass_utils import run_bass_kernel_spmd

F32 = mybir.dt.float32
F32R = mybir.dt.float32r
BF16 = mybir.dt.bfloat16

B, T, C = 2, 2048, 1024
H, HKV, D = 16, 4, 64
HALF = D // 2  # 32
GQ = H // HKV  # 4 q heads per group
FQ = GQ * D    # 256 q features per group
FPROJ = FQ + 2 * D  # 384: q(256) + k(64) + v(64)
NT = T // 512  # 4 row blocks of 512
KT = C // 128  # 8 contraction tiles
MT = FPROJ // 128  # 3 output row tiles (q01, q23, kv)
NEG = -1.0e9


def _split_excess_waits(nc, max_waits=1):
    """walrus here encodes at most one sync-wait per instruction; hoist the
    rest into standalone EventSemaphore instructions (raw-bass encoding)."""
    n = 0
    for fn in nc.m.functions:
        for bb in fn.blocks:
            new = []
            changed = False
            for inst in bb.instructions:
                si = inst.sync_info
                if si is not None and len(si.on_wait) > max_waits:
                    waits = list(si.on_wait)
                    for j, w in enumerate(waits[max_waits:]):
                        ev = mybir.InstEventSemaphore(
                            name=f"{inst.name}-ws{j}",
                            engine=inst.engine,
                            ins=[],
                            outs=[],
                            sync_info=mybir.SyncInfo(on_wait=[w], on_update=[]),
                        )
                        new.append(ev)
                        n += 1
                    inst.sync_info = mybir.SyncInfo(
                        on_wait=waits[:max_waits], on_update=list(si.on_update)
                    )
                    changed = True
                new.append(inst)
            if changed:
                bb.instructions = new
    return n


def _build():
    nc = bass.Bass()
    xt_d = nc.dram_tensor("xt", [C, T], BF16, kind="ExternalInput")
    wproj_d = nc.dram_tensor("wproj", [C, FPROJ], BF16, kind="ExternalInput")
    wo_d = nc.dram_tensor("wo", [FQ, C], BF16, kind="ExternalInput")
    atab_d = nc.dram_tensor("atab", [128, T], BF16, kind="ExternalInput")
    btab_d = nc.dram_tensor("btab", [128, T], BF16, kind="ExternalInput")
    trib_d = nc.dram_tensor("trib", [128, 128], BF16, kind="ExternalInput")
    identr_d = nc.dram_tensor("identr", [64, 64], BF16, kind="ExternalInput")
    ones65_d = nc.dram_tensor("ones65", [65, 64], F32, kind="ExternalInput")
    zeros_d = nc.dram_tensor("zeros64", [64, T], BF16, kind="ExternalInput")
    y_d = nc.dram_tensor("y", [T, C], F32, kind="ExternalOutput")

    xt_r = xt_d.rearrange("(ko p) t -> p ko t", p=128)
    wproj_r = wproj_d.rearrange("(ko p) f -> p ko f", p=128)

    with TileContext(nc) as tc:
        from contextlib import ExitStack

        with ExitStack() as ctx:
            const = ctx.enter_context(tc.tile_pool(name="const", bufs=1))
            pers = ctx.enter_context(tc.tile_pool(name="pers", bufs=1))
            # --- constants ---
            wproj_sb = const.tile([128, KT, FPROJ], BF16)
            wo_sb = const.tile([128, 2, C], BF16)
            atab = const.tile([128, T], BF16)
            btab = const.tile([128, T], BF16)
            trib = const.tile([128, 128], BF16)
            identr = const.tile([128, 64], BF16)
            ones65 = const.tile([65, 64], F32R)

            # --- persistent activations ---
            qr = [pers.tile([128, T], BF16, name=f"qr{i}") for i in range(2)]
            # k^T zero-padded to 128 contraction rows: kr0 = [k; 0] for even
            # heads, kr1 = [0; k] for odd heads -> S matmuls engage the full
            # PE array while the zero half kills the other head's q rows.
            kr0 = pers.tile([128, T], BF16)
            kr1 = pers.tile([128, T], BF16)
            vsb = pers.tile([128, T // 128, 65], BF16)  # v natural + ones col
            opk = pers.tile([128, 2, T], BF16)  # packed normalized O^T for Wo

            xpool = ctx.enter_context(tc.tile_pool(name="xp", bufs=2))
            tmp = ctx.enter_context(tc.tile_pool(name="tmp", bufs=2))
            ppool = ctx.enter_context(tc.tile_pool(name="pp", bufs=4))
            o65pool = ctx.enter_context(tc.tile_pool(name="o65p", bufs=2))
            rcpool = ctx.enter_context(tc.tile_pool(name="rc", bufs=2))
            ypool = ctx.enter_context(tc.tile_pool(name="yp", bufs=2))
            # PSUM: pp(2) + s(2x2) + o(2) = 8 banks
            pp_ps = ctx.enter_context(
                tc.tile_pool(name="ppps", bufs=2, space="PSUM")
            )
            spool = ctx.enter_context(
                tc.tile_pool(name="sps", bufs=2, space="PSUM")
            )
            opool = ctx.enter_context(
                tc.tile_pool(name="ops", bufs=2, space="PSUM")
            )

            xrows = {}

            def emit_x_dma(n):
                xr = xpool.tile([128, KT, 512], BF16, tag="x", name=f"x{n}")
                xrows[n] = xr
                if n == 0:
                    return  # row 0 loads per-k, interleaved with wproj
                for half in range(2):
                    ks = slice(4 * half, 4 * half + 4)
                    nc.sync.dma_start(
                        xr[:, ks], xt_r[:, ks, bass.ts(n, 512)]
                    )

            def emit_preamble_dma():
                emit_x_dma(0)
                xr = xrows[0]
                nc.gpsimd.memset(vsb[:, :, 64], 1.0)
                for k in range(2):
                    nc.sync.dma_start(wproj_sb[:, k], wproj_r[:, k])
                    nc.sync.dma_start(xr[:, k], xt_r[:, k, bass.ts(0, 512)])
                nc.sync.dma_start(identr[64:128, :], identr_d[:])
                nc.sync.dma_start(atab[:], atab_d[:])
                nc.sync.dma_start(btab[:], btab_d[:])
                for k in range(2, KT):
                    nc.sync.dma_start(wproj_sb[:, k], wproj_r[:, k])
                    nc.sync.dma_start(xr[:, k], xt_r[:, k, bass.ts(0, 512)])
                nc.sync.dma_start(kr0[64:128, :], zeros_d[:])
                nc.sync.dma_start(kr1[0:64, :], zeros_d[:])
                nc.sync.dma_start(trib[:], trib_d[:])
                nc.sync.dma_start(ones65[:], ones65_d[:].bitcast(F32R))
                nc.sync.dma_start(
                    wo_sb[:], wo_d.rearrange("(ko p) c -> p ko c", p=128)
                )

            def emit_proj(n):
                """projections + RoPE for token block n -> qr/kr/vsb cols."""
                xr = xrows[n]
                for m in range(MT):
                    ps = pp_ps.tile([128, 512], F32, tag="pp", name=f"ps{m}")
                    for k in range(KT):
                        nc.tensor.matmul(
                            ps[:],
                            wproj_sb[:, k, bass.ts(m, 128)],
                            xr[:, k],
                            start=(k == 0),
                            stop=(k == KT - 1),
                        )
                    rows = 128 if m < 2 else 64
                    qt_t = tmp.tile(
                        [128, 512], BF16, tag="qt", name="qt", bufs=3
                    )
                    plain = qt_t[:]
                    nc.vector.tensor_copy(plain, ps[:])
                    # rotate-half partition swap (p <-> p^32) via SBUF DMAs
                    qs = tmp.tile([128, 512], BF16, tag="qs", name=f"qs{m}")
                    nc.sync.dma_start(qs[0:32], plain[32:64])
                    nc.sync.dma_start(qs[32:64], plain[0:32])
                    if m < 2:
                        nc.sync.dma_start(qs[64:96], plain[96:128])
                        nc.sync.dma_start(qs[96:128], plain[64:96])
                    t1 = tmp.tile([128, 512], BF16, tag="t1")
                    nc.vector.tensor_tensor(
                        t1[0:rows],
                        plain[0:rows],
                        atab[0:rows, bass.ts(n, 512)],
                        mybir.AluOpType.mult,
                    )
                    t2 = tmp.tile([128, 512], BF16, tag="t2")
                    nc.vector.tensor_tensor(
                        t2[0:rows],
                        qs[0:rows],
                        btab[0:rows, bass.ts(n, 512)],
                        mybir.AluOpType.mult,
                    )
                    dest = qr[m] if m < 2 else kr0
                    nc.gpsimd.tensor_tensor(
                        dest[0:rows, bass.ts(n, 512)],
                        t1[0:rows],
                        t2[0:rows],
                        mybir.AluOpType.add,
                    )
                    if m == 2:
                        # duplicate k^T into kr1 rows 64:128
                        nc.vector.tensor_copy(
                            kr1[64:128, bass.ts(n, 512)],
                            kr0[0:64, bass.ts(n, 512)],
                        )
                        # v^T -> v natural (PE transpose per 128-token block)
                        for tt in range(4 * n, 4 * n + 4):
                            vt_ps = pp_ps.tile(
                                [128, 64], BF16, tag="pp", name="vt"
                            )
                            nc.tensor.transpose(
                                vt_ps[:],
                                plain[64:128, bass.ts(tt - 4 * n, 128)],
                                identr[64:128, :],
                            )
                            nc.vector.tensor_copy(vsb[:, tt, 0:64], vt_ps[:])

            def emit_attn_pair(n, hp, o65b):
                """S/exp/O for one head pair of query row n."""
                jq = n
                nkb = 4 * (jq + 1)
                qtile = qr[hp]
                heads = (2 * hp, 2 * hp + 1)
                o_ps = {
                    h: opool.tile([65, 512], F32, tag="o", name=f"o{h}")
                    for h in heads
                }
                pend = None
                for ksb in range(nkb // 2):
                    regions = []
                    for jk in range(2):
                        kb = 2 * ksb + jk
                        j = kb - 4 * jq
                        col0 = max(j, 0) * 128
                        regions.append((jk, col0, kb))
                    s_ps = {}
                    p_sb = {}
                    for h in heads:
                        s_ps[h] = spool.tile(
                            [128, 1024], F32, tag="s", name=f"s{h}"
                        )
                        p_sb[h] = ppool.tile(
                            [128, 1024], BF16, tag="p", name=f"pb{h}"
                        )
                    for jk, col0, kb in regions:
                        for h in heads:
                            krt = kr0 if h % 2 == 0 else kr1
                            nc.tensor.matmul(
                                s_ps[h][:, jk * 512 + col0 : jk * 512 + 512],
                                krt[:, bass.ts(kb, 128)],
                                qtile[:, jq * 512 + col0 : jq * 512 + 512],
                                start=True,
                                stop=True,
                            )
                    for h in heads:
                        if regions[0][1] == 0 and regions[1][1] == 0:
                            nc.scalar.activation(
                                p_sb[h][:],
                                s_ps[h][:],
                                mybir.ActivationFunctionType.Exp,
                                scale=0.125,
                            )
                        else:
                            for jk, col0, kb in regions:
                                nc.scalar.activation(
                                    p_sb[h][:, jk * 512 + col0 : jk * 512 + 512],
                                    s_ps[h][:, jk * 512 + col0 : jk * 512 + 512],
                                    mybir.ActivationFunctionType.Exp,
                                    scale=0.125,
                                )
                    for h in heads:
                        for jk, col0, kb in regions:
                            if kb - 4 * jq >= 0:
                                nc.gpsimd.tensor_tensor(
                                    p_sb[h][
                                        :,
                                        jk * 512 + col0 : jk * 512 + col0 + 128,
                                    ],
                                    p_sb[h][
                                        :,
                                        jk * 512 + col0 : jk * 512 + col0 + 128,
                                    ],
                                    trib[:],
                                    mybir.AluOpType.mult,
                                )
                    # O matmuls run one K-step behind S so the PE queue
                    # never head-of-line blocks on the scalar exps.
                    if pend is not None:
                        for h in heads:
                            for jk, col0, kb in pend[0]:
                                nc.tensor.matmul(
                                    o_ps[h][:, col0:512],
                                    vsb[:, kb, :],
                                    pend[1][h][
                                        :, jk * 512 + col0 : jk * 512 + 512
                                    ],
                                    start=(kb == 0),
                                    stop=(kb == nkb - 1),
                                )
                    pend = (regions, p_sb)
                for h in heads:
                    for jk, col0, kb in pend[0]:
                        nc.tensor.matmul(
                            o_ps[h][:, col0:512],
                            vsb[:, kb, :],
                            pend[1][h][:, jk * 512 + col0 : jk * 512 + 512],
                            start=(kb == 0),
                            stop=(kb == nkb - 1),
                        )
                for h in heads:
                    nc.vector.tensor_copy(o65b[:, h, :], o_ps[h][:])
                # batched 1/rowsum for the pair: Ln + Exp(-x) on the scalar
                # engine (both live in the natural_log_exp act table).
                lnd = rcpool.tile(
                    [1, 2, 512], F32, tag="lnd", name=f"ln{jq}_{hp}", bufs=2
                )
                nc.scalar.activation(
                    lnd[0:1, :, :],
                    o65b[64:65, 2 * hp : 2 * hp + 2, :].bitcast(F32),
                    mybir.ActivationFunctionType.Ln,
                )
                rc = rcpool.tile(
                    [1, 2, 512], F32R, tag="rc", name=f"rc{jq}_{hp}", bufs=2
                )
                nc.scalar.activation(
                    rc[0:1, :, :],
                    lnd[0:1, :, :],
                    mybir.ActivationFunctionType.Exp,
                    scale=-1.0,
                )
                return rc

            def emit_norm_pair(n, hp, o65b, rc):
                """broadcast 1/rowsum via PE, apply via DVE -> opk (bf16)."""
                jq = n
                for hh in range(2):
                    h = 2 * hp + hh
                    bc_ps = pp_ps.tile([64, 512], F32, tag="pp", name=f"bc{h}")
                    nc.tensor.matmul(
                        bc_ps[:],
                        ones65[0:1, :],
                        rc[0:1, hh, :],
                        start=True,
                        stop=True,
                    )
                    nc.vector.tensor_tensor(
                        opk[(h % 2) * 64 : (h % 2) * 64 + 64, h // 2, bass.ts(jq, 512)],
                        o65b[0:64, h, :].bitcast(F32),
                        bc_ps[:],
                        mybir.AluOpType.mult,
                    )

            def emit_wo(n):
                """Wo projection + writeout for the row's 4 token blocks."""
                for t in range(4 * n, 4 * n + 4):
                    y_sb = ypool.tile([128, C], F32, tag="y", name="ysb")
                    for nn in range(2):
                        wps = pp_ps.tile([128, 512], F32, tag="pp", name="wps")
                        for k in range(2):
                            nc.tensor.matmul(
                                wps[:],
                                opk[:, k, bass.ts(t, 128)],
                                wo_sb[:, k, bass.ts(nn, 512)],
                                start=(k == 0),
                                stop=(k == 1),
                            )
                        nc.vector.tensor_copy(y_sb[:, bass.ts(nn, 512)], wps[:])
                    nc.sync.dma_start(y_d[bass.ts(t, 128), :], y_sb[:])

            emit_preamble_dma()
            emit_proj(0)
            for n in range(NT):
                o65b = o65pool.tile(
                    [65, 4, 512], F32R, tag="o65", name=f"o65_{n}"
                )
                rc0 = emit_attn_pair(n, 0, o65b)
                rc1 = emit_attn_pair(n, 1, o65b)
                # proj(n+1) fills the PE while the Ln/Exp chains run on the
                # scalar engine; the broadcasts follow.
                if n + 1 < NT:
                    emit_x_dma(n + 1)
                    emit_proj(n + 1)
                emit_norm_pair(n, 0, o65b, rc0)
                emit_norm_pair(n, 1, o65b, rc1)
                emit_wo(n)

    _split_excess_waits(nc)
    return nc


_NC_CACHE = None


def _get_nc():
    global _NC_CACHE
    if _NC_CACHE is None:
        _NC_CACHE = _build()
    return _NC_CACHE


def _host_prep(x, cos, sin, Wq, Wk, Wv, Wo):
    import ml_dtypes

    cos2 = np.asarray(cos, np.float32).reshape(T, HALF)  # [T, 32]
    sin2 = np.asarray(sin, np.float32).reshape(T, HALF)
    atab = np.tile(cos2.T, (4, 1)).astype(ml_dtypes.bfloat16)  # [128, T]
    btab = np.tile(np.vstack([sin2.T, -sin2.T]), (2, 1)).astype(
        ml_dtypes.bfloat16
    )
    k_i = np.arange(128)[:, None]
    q_i = np.arange(128)[None, :]
    trib = np.where(k_i > q_i, np.float32(0.0), np.float32(1.0)).astype(
        ml_dtypes.bfloat16
    )
    identr = np.eye(64, dtype=ml_dtypes.bfloat16)
    ones65 = np.ones((65, 64), np.float32)
    zeros64 = np.zeros((64, T), ml_dtypes.bfloat16)

    in_maps = []
    for core in range(8):
        b, g = core // 4, core % 4
        xt = np.ascontiguousarray(
            np.asarray(x[b], np.float32).T.astype(ml_dtypes.bfloat16)
        )  # [C, T]
        wproj = np.ascontiguousarray(
            np.concatenate(
                [
                    Wq[:, g * FQ : (g + 1) * FQ],
                    Wk[:, g * D : (g + 1) * D],
                    Wv[:, g * D : (g + 1) * D],
                ],
                axis=1,
            ).astype(ml_dtypes.bfloat16)
        )
        wo = np.ascontiguousarray(
            Wo[g * FQ : (g + 1) * FQ, :].astype(ml_dtypes.bfloat16)
        )
        in_maps.append(
            {
                "xt": xt,
                "wproj": wproj,
                "wo": wo,
                "atab": atab,
                "btab": btab,
                "trib": trib,
                "identr": identr,
                "ones65": ones65,
                "zeros64": zeros64,
            }
        )
    return in_maps


def kernel(x, cos, sin, Wq, Wk, Wv, Wo, _want_trace=False, _trace_kwargs=None):
    nc = _get_nc()
    in_maps = _host_prep(x, cos, sin, Wq, Wk, Wv, Wo)
    kw = {}
    if _want_trace:
        kw = dict(trace=True, **(_trace_kwargs or {}))
    res = run_bass_kernel_spmd(nc, in_maps, list(range(8)), **kw)
    y = np.zeros((B, T, C), np.float32)
    for core in range(8):
        b = core // 4
        y[b] += res.results[core]["y"]
    if _want_trace:
        kernel.last_result = res
    return y
